# revision 14
# baseline (speedup 1.0000x reference)
"""EnhancedGCN on 8 Trainium2 NeuronCores (Bass/Tile, SPMD).

Strategy: 1D node partition (6250 nodes/core, padded to 6272). Step-0's
gather table xws0 = dis * (in_feat @ W2) (W2 = conv_w @ lin_w pre-fused
host-side) is computed REPLICATED on every core straight into local DRAM —
no step-0 collective at all. The lin bias term is rank-1 and folds into the
epilogue through a host-precomputed per-node scalar q = dis_t * sum_dis.
Step-1 publishes xws1 = dis * (hB @ conv_w.T) via 4 chunked AllGathers
(Shared outputs, triggered from the Sync engine so they never queue behind
gather calls). Per-edge messages are fetched with dma_gather (4 SWDGE
queues, round-robin assignment, super-0-leading prefetch) and reduced into
per-target sums with 0/1 selection-matrix matmuls accumulating in PSUM
(self-loops enter as an identity-matmul block). Edge weights
ew = dis[t]*dis[s] are separable: dis[s] pre-scales the table, dis[t]
post-scales the message sum. Host-side work is limited to graph-structure
prep (sorting edges into target windows, block padding, int16 index
streams) and weight transposes/fusion.
"""
import sys

sys.path.insert(0, "/opt/trn_rl_repo")

import numpy as np
import ml_dtypes

import concourse.bass as bass
import concourse.bacc as bacc
import concourse.tile as tile
import concourse.mybir as mybir
from concourse.bass_utils import run_bass_kernel_spmd
from concourse.masks import make_identity

BF16 = ml_dtypes.bfloat16
N, IN, H = 50000, 256, 128
NCORES = 8
NPC = N // NCORES  # 6250
NW = (NPC + 127) // 128  # 49
PADN = NW * 128  # 6272
LN_EPS = 1e-5
NGRP = (NW + 3) // 4  # 13 groups of 4 windows

# table chunks (windows per chunk) and the two gather super-streams
CHUNK_W = [12, 12, 12, 13]
CHUNK_W0 = [0, 12, 24, 36, 49]
NCHUNK = 4
NSUP = 2  # chunks 0+1 -> super 0 (windows 0..23), chunks 2+3 -> super 1
SUP_OF_CHUNK = [0, 0, 1, 1]
# table row layout per super: [chunkA: 8 ranks x szA | chunkB: 8 ranks x szB]
CHUNK_SZ = [cw * 128 for cw in CHUNK_W]
TBL_ROWS = [8 * (CHUNK_SZ[0] + CHUNK_SZ[1]), 8 * (CHUNK_SZ[2] + CHUNK_SZ[3])]
TB_OFF = [0, 8 * CHUNK_SZ[0], 0, 8 * CHUNK_SZ[2]]

F32 = mybir.dt.float32
BF = mybir.dt.bfloat16
I16 = mybir.dt.int16
AX = mybir.AluOpType
AF = mybir.ActivationFunctionType


def _bcast_mid(ap, n):
    """[128, F] AP -> [128, n, F] with stride-0 middle dim."""
    a = ap.copy()
    a.ap = [a.ap[0], [0, n]] + a.ap[1:]
    return a


def _r3(ap, f):
    return ap.rearrange("p (w f) -> p w f", f=f)


def _wrap_idx(idx):
    """flat idx [n] (n % 16 == 0) -> [128, n/16] int16 wrapped + replicated."""
    n = len(idx)
    t = idx.reshape(n // 16, 16).T.astype(np.int16)
    return np.tile(t, (8, 1))


def _prep_graph(row, col):
    """Graph-structure-only preprocessing (row/col ints)."""
    deg = np.bincount(row, minlength=N).astype(np.float64) + 1.0
    dis_f = 1.0 / np.sqrt(deg)
    dinv_f = 1.0 / deg
    # q_t = dis_t * (sum_{s in N(t)} dis_s + dis_t): rank-1 lin-bias epilogue
    csum = np.bincount(row, weights=dis_f[col], minlength=N) + dis_f
    q_f = dis_f * csum

    core = row // NPC
    src_core = col // NPC
    src_off = col % NPC
    src_w = src_off >> 7
    src_chunk = np.digitize(src_w, CHUNK_W0[1:4])  # 0..3
    src_sup = (src_chunk >= 2).astype(np.int64)
    base = np.asarray(TB_OFF)[src_chunk]
    csz = np.asarray(CHUNK_SZ)[src_chunk]
    w0 = np.asarray(CHUNK_W0)[src_chunk] * 128
    src_idx = base + src_core * csz + (src_off - w0)

    per_core = []
    counts = np.zeros((NCORES, NW, NSUP), np.int64)
    for k in range(NCORES):
        m = core == k
        tgt = (row[m] - k * NPC).astype(np.int64)
        sidx = src_idx[m]
        ssup = src_sup[m]
        w = tgt >> 7
        order = np.argsort(w, kind="stable")
        tgt, sidx, ssup, w = tgt[order], sidx[order], ssup[order], w[order]
        ents = []
        bounds = np.searchsorted(w, np.arange(NW + 1))
        for wi in range(NW):
            sl = slice(bounds[wi], bounds[wi + 1])
            s_w, t_w, u_w = sidx[sl], tgt[sl] - (wi << 7), ssup[sl]
            by_sup = []
            for s in range(NSUP):
                mm = u_w == s
                by_sup.append((s_w[mm], t_w[mm]))
                counts[k, wi, s] = int(mm.sum())
            ents.append(by_sup)
        per_core.append(ents)

    B = np.ceil(counts.max(axis=0) / 128).astype(np.int64)  # [NW, NSUP]
    NBLK = B.sum(axis=0).astype(np.int64)  # per super
    gpos = np.zeros((NSUP, NGRP + 1), np.int64)
    for s in range(NSUP):
        pref = np.concatenate([[0], np.cumsum(B[:, s])])
        for g in range(NGRP + 1):
            gpos[s, g] = pref[min(g * 4, NW)]

    rng = np.random.default_rng(12345)
    idx_streams = [np.empty((NCORES, int(NBLK[s]) * 128), np.int64) for s in range(NSUP)]
    for s in range(NSUP):
        idx_streams[s][:] = rng.integers(0, TBL_ROWS[s], idx_streams[s].shape)
    tlocs = [np.full((NCORES, 128, int(NBLK[s])), -1.0, np.float32) for s in range(NSUP)]

    for k in range(NCORES):
        pos = [0] * NSUP
        for wi in range(NW):
            for s in range(NSUP):
                s_w, t_w = per_core[k][wi][s]
                n = len(s_w)
                p = pos[s]
                idx_streams[s][k, p * 128 : p * 128 + n] = s_w
                j = np.arange(n)
                tlocs[s][k, j % 128, p + j // 128] = t_w
                pos[s] += int(B[wi, s])

    # wrap idx per half-call segment (aligned to group boundaries, split in two)
    idx_w = [None] * NSUP
    for s in range(NSUP):
        per_core_w = [[] for _ in range(NCORES)]
        for g in range(NGRP):
            b0, b1 = int(gpos[s, g]), int(gpos[s, g + 1])
            mid = b0 + (b1 - b0 + 1) // 2
            for (h0_, h1_) in ((b0, mid), (mid, b1)):
                if h1_ > h0_:
                    for k in range(NCORES):
                        per_core_w[k].append(
                            _wrap_idx(idx_streams[s][k, h0_ * 128 : h1_ * 128])
                        )
        idx_w[s] = np.stack([np.concatenate(x, axis=1) for x in per_core_w])

    dis_cols = np.zeros((NCORES, 128, NW), np.float32)
    dinv_cols = np.ones((NCORES, 128, NW), np.float32)
    q_cols = np.zeros((NCORES, 128, NW), np.float32)
    dis_full = np.zeros((128, NCORES * NW), np.float32)
    for k in range(NCORES):
        v = np.zeros(PADN, np.float64)
        v[:NPC] = dis_f[k * NPC : (k + 1) * NPC]
        dis_cols[k] = v.reshape(NW, 128).T
        dis_full[:, k * NW : (k + 1) * NW] = dis_cols[k]
        u = np.ones(PADN, np.float64)
        u[:NPC] = dinv_f[k * NPC : (k + 1) * NPC]
        dinv_cols[k] = u.reshape(NW, 128).T
        qv = np.zeros(PADN, np.float64)
        qv[:NPC] = q_f[k * NPC : (k + 1) * NPC]
        q_cols[k] = qv.reshape(NW, 128).T

    return dict(
        B=B,
        NBLK=NBLK,
        gpos=gpos,
        idx_w=idx_w,
        tlocs=[t.astype(BF16) for t in tlocs],
        dis_cols=dis_cols,
        dinv_cols=dinv_cols,
        q_cols=q_cols,
        dis_full=dis_full,
    )


def _build(B, gpos, NBLK):
    nc = bacc.Bacc("TRN2", target_bir_lowering=False, debug=False, num_swdge_queues=4)

    ift = nc.dram_tensor("ift", [IN, NCORES * PADN], BF, kind="ExternalInput")
    iftl = nc.dram_tensor("iftl", [IN, PADN], BF, kind="ExternalInput")
    lin_wT = nc.dram_tensor("lin_wT", [IN, H], BF, kind="ExternalInput")
    w2T = nc.dram_tensor("w2T", [IN, H], BF, kind="ExternalInput")
    conv_wT = nc.dram_tensor("conv_wT", [H, H], BF, kind="ExternalInput")
    consts = nc.dram_tensor("consts", [128, 6 * H], F32, kind="ExternalInput")
    iota_in = nc.dram_tensor("iota", [128, 128], BF, kind="ExternalInput")
    discols = nc.dram_tensor("discols", [128, NW], F32, kind="ExternalInput")
    disfull = nc.dram_tensor("disfull", [128, NCORES * NW], F32, kind="ExternalInput")
    dinvcols = nc.dram_tensor("dinvcols", [128, NW], F32, kind="ExternalInput")
    qcols = nc.dram_tensor("qcols", [128, NW], F32, kind="ExternalInput")
    idx_t = [
        nc.dram_tensor(f"idx{s}", [128, int(NBLK[s]) * 8], I16, kind="ExternalInput")
        for s in range(NSUP)
    ]
    tloc_t = [
        nc.dram_tensor(f"tloc{s}", [128, int(NBLK[s])], BF, kind="ExternalInput")
        for s in range(NSUP)
    ]
    out_ext = nc.dram_tensor("out", [PADN, H], F32, kind="ExternalOutput")

    def ws(w):
        return slice(w * 128, (w + 1) * 128)

    # per-(stream, group) half-call boundaries + column offset into wrapped idx
    halves = {}
    for s in range(NSUP):
        off = 0
        for g in range(NGRP):
            b0, b1 = int(gpos[s, g]), int(gpos[s, g + 1])
            mid = b0 + (b1 - b0 + 1) // 2
            hs = []
            for (h0_, h1_) in ((b0, mid), (mid, b1)):
                hs.append((h0_, h1_, off))
                off += (h1_ - h0_) * 8
            halves[(s, g)] = hs
    gmax = max(h1 - h0 for v in halves.values() for (h0, h1, _) in v)
    wmax = int(B.sum(axis=1).max())
    pw = np.concatenate([np.zeros((1, NSUP), np.int64), np.cumsum(B, axis=0)], axis=0)

    with tile.TileContext(nc) as tc:
        with (
            tc.tile_pool(name="const", bufs=1) as cpool,
            tc.tile_pool(name="state", bufs=1) as spool,
            tc.tile_pool(name="iftp", bufs=2) as ipool,
            tc.tile_pool(name="lftp", bufs=4) as lpool,
            tc.tile_pool(name="ht", bufs=4) as hpool,
            tc.tile_pool(name="gath", bufs=12) as gpool,
            tc.tile_pool(name="wp", bufs=5) as wpool,
            tc.tile_pool(name="tmp", bufs=1) as tpool,
            tc.tile_pool(name="psA", bufs=4, space="PSUM") as psA,
            tc.tile_pool(name="psM", bufs=4, space="PSUM") as psM,
            tc.tile_pool(name="dram", bufs=1, space="DRAM") as dpool,
        ):
            identf = cpool.tile([128, 128], F32)
            make_identity(nc, identf[:])
            identb = cpool.tile([128, 128], BF)
            nc.vector.tensor_copy(out=identb[:], in_=identf[:])
            cst = cpool.tile([128, 6 * H], F32)
            nc.sync.dma_start(out=cst[:], in_=consts[:])
            linb, rootr, convbr, g1r, b1r, b2r = (
                cst[:, i * H : (i + 1) * H] for i in range(6)
            )
            iot = cpool.tile([128, 128], BF)
            nc.sync.dma_start(out=iot[:], in_=iota_in[:])
            cw = cpool.tile([128, H], BF)
            nc.sync.dma_start(out=cw[:], in_=conv_wT[:])
            lw0 = cpool.tile([128, H], BF)
            nc.sync.dma_start(out=lw0[:], in_=lin_wT[0:128, :])
            lw1 = cpool.tile([128, H], BF)
            nc.sync.dma_start(out=lw1[:], in_=lin_wT[128:256, :])
            w20 = cpool.tile([128, H], BF)
            nc.sync.dma_start(out=w20[:], in_=w2T[0:128, :])
            w21 = cpool.tile([128, H], BF)
            nc.sync.dma_start(out=w21[:], in_=w2T[128:256, :])
            dic = cpool.tile([128, NW], F32)
            nc.sync.dma_start(out=dic[:], in_=discols[:])
            dfl = cpool.tile([128, NCORES * NW], F32)
            nc.sync.dma_start(out=dfl[:], in_=disfull[:])
            dvc = cpool.tile([128, NW], F32)
            nc.sync.dma_start(out=dvc[:], in_=dinvcols[:])
            qcl = cpool.tile([128, NW], F32)
            nc.sync.dma_start(out=qcl[:], in_=qcols[:])
            tl_sb = []
            idx_sb = []
            for s in range(NSUP):
                t = cpool.tile([128, int(NBLK[s])], BF, name=f"tl{s}")
                nc.sync.dma_start(out=t[:], in_=tloc_t[s][:])
                tl_sb.append(t)
                t2 = cpool.tile([128, int(NBLK[s]) * 8], I16, name=f"ix{s}")
                nc.sync.dma_start(out=t2[:], in_=idx_t[s][:])
                idx_sb.append(t2)

            h0 = spool.tile([128, PADN], F32, tag="h0")
            hA = spool.tile([128, PADN], F32, tag="hA")
            hB = spool.tile([128, PADN], F32, tag="hB")
            xws0 = spool.tile([128, PADN], BF, tag="xws0")
            xws1 = spool.tile([128, PADN], BF, tag="xws1")

            # ---- step-0 table: replicated full-graph xws0 into local DRAM ----
            tb0 = [
                dpool.tile([TBL_ROWS[0], H], BF, tag="tb0_0", name="tb0A"),
                dpool.tile([TBL_ROWS[1], H], BF, tag="tb0_1", name="tb0B"),
            ]
            for c in range(NCHUNK):
                w0c, w1c = CHUNK_W0[c], CHUNK_W0[c + 1]
                cwn = w1c - w0c
                csz = CHUNK_SZ[c]
                sup = SUP_OF_CHUNK[c]
                for k in range(NCORES):
                    col0 = k * PADN + w0c * 128
                    i0 = ipool.tile([128, CHUNK_SZ[3]], BF, tag="ifta")
                    nc.sync.dma_start(out=i0[:, :csz], in_=ift[0:128, col0 : col0 + csz])
                    i1 = ipool.tile([128, CHUNK_SZ[3]], BF, tag="iftb")
                    nc.sync.dma_start(out=i1[:, :csz], in_=ift[128:256, col0 : col0 + csz])
                    for wq in range(cwn):
                        xp = psA.tile([128, 128], F32, tag="ps128")
                        nc.tensor.matmul(
                            xp[:], lhsT=i0[:, ws(wq)], rhs=w20[:], start=True, stop=False
                        )
                        nc.tensor.matmul(
                            xp[:], lhsT=i1[:, ws(wq)], rhs=w21[:], start=False, stop=True
                        )
                        # i1's window is dead after its matmul: reuse it as the
                        # bf16 staging for the table write
                        nc.scalar.activation(
                            out=i1[:, ws(wq)],
                            in_=xp[:],
                            func=AF.Copy,
                            scale=dfl[:, k * NW + w0c + wq : k * NW + w0c + wq + 1],
                        )
                    nc.sync.dma_start(
                        out=tb0[sup][:][
                            TB_OFF[c] + k * csz : TB_OFF[c] + (k + 1) * csz, :
                        ].rearrange("(w p) f -> p w f", p=128),
                        in_=_r3(i1[:, :csz], 128),
                    )

            # ---- local shard: h0 (lin) and xws0 (for self-loop matmuls) ----
            for w in range(NW):
                i0 = lpool.tile([128, 128], BF, tag="lfta")
                nc.sync.dma_start(out=i0[:], in_=iftl[0:128, ws(w)])
                i1 = lpool.tile([128, 128], BF, tag="lftb")
                nc.sync.dma_start(out=i1[:], in_=iftl[128:256, ws(w)])
                hp = psA.tile([128, 128], F32, tag="ps128")
                nc.tensor.matmul(hp[:], lhsT=i0[:], rhs=lw0[:], start=True, stop=False)
                nc.tensor.matmul(hp[:], lhsT=i1[:], rhs=lw1[:], start=False, stop=True)
                nc.vector.tensor_tensor(out=h0[:, ws(w)], in0=hp[:], in1=linb, op=AX.add)
                xp = psA.tile([128, 128], F32, tag="ps128")
                nc.tensor.matmul(xp[:], lhsT=i0[:], rhs=w20[:], start=True, stop=False)
                nc.tensor.matmul(xp[:], lhsT=i1[:], rhs=w21[:], start=False, stop=True)
                nc.scalar.activation(
                    out=xws0[:, ws(w)], in_=xp[:], func=AF.Copy, scale=dic[:, w : w + 1]
                )

            ctxs = {
                0: dict(tb=tb0, call_tiles={}, w_tiles={}, blkpos=[0] * NSUP),
            }
            qrr = [0]  # round-robin SWDGE queue assignment

            def emit_publish(c):
                """Step-1: LN + xws1 for chunk c + publish + AllGather (Shared out)."""
                if 1 not in ctxs:
                    tbA = dpool.tile([TBL_ROWS[0], H], BF, tag="tb1_0", name="tb1A")
                    tbB = dpool.tile([TBL_ROWS[1], H], BF, tag="tb1_1", name="tb1B")
                    ctxs[1] = dict(tb=[tbA, tbB], call_tiles={}, w_tiles={}, blkpos=[0] * NSUP)
                ctx = ctxs[1]
                w0c, w1c = CHUNK_W0[c], CHUNK_W0[c + 1]
                # residual + layernorm + relu -> hB for this chunk's windows
                for g in range(w0c, w1c, 4):
                    gw = min(4, w1c - g)
                    sl = slice(g * 128, (g + gw) * 128)
                    X_t = tpool.tile([128, 4 * 128], F32, tag="ln_X")
                    X = X_t[:, : gw * 128]
                    Y_t = tpool.tile([128, 4 * 128], F32, tag="ln_Y")
                    Y = Y_t[:, : gw * 128]
                    nc.vector.tensor_tensor(out=X, in0=hA[:, sl], in1=h0[:, sl], op=AX.add)
                    mu_t = tpool.tile([128, 4], F32, tag="ln_mu")
                    mu = mu_t[:, :gw]
                    nc.vector.tensor_reduce(out=mu, in_=_r3(X, 128), axis=mybir.AxisListType.X, op=AX.add)
                    nc.vector.tensor_scalar_mul(out=mu, in0=mu, scalar1=1.0 / 128.0)
                    nc.vector.tensor_tensor(out=Y, in0=X, in1=X, op=AX.mult)
                    var_t = tpool.tile([128, 4], F32, tag="ln_var")
                    var = var_t[:, :gw]
                    nc.vector.tensor_reduce(out=var, in_=_r3(Y, 128), axis=mybir.AxisListType.X, op=AX.add)
                    mm_t = tpool.tile([128, 4], F32, tag="ln_mm")
                    mm = mm_t[:, :gw]
                    nc.vector.tensor_tensor(out=mm, in0=mu, in1=mu, op=AX.mult)
                    nc.vector.tensor_scalar(
                        out=var, in0=var, scalar1=1.0 / 128.0, scalar2=LN_EPS, op0=AX.mult, op1=AX.add
                    )
                    nc.vector.tensor_tensor(out=var, in0=var, in1=mm, op=AX.subtract)
                    sd_t = tpool.tile([128, 4], F32, tag="ln_sd")
                    sd = sd_t[:, :gw]
                    nc.scalar.activation(out=sd, in_=var, func=AF.Sqrt)
                    rstd_t = tpool.tile([128, 4], F32, tag="ln_rs")
                    rstd = rstd_t[:, :gw]
                    nc.vector.reciprocal(out=rstd, in_=sd)
                    mb_t = tpool.tile([128, 4], F32, tag="ln_mb")
                    mb = mb_t[:, :gw]
                    nc.vector.tensor_tensor(out=mb, in0=mu, in1=rstd, op=AX.mult)
                    nc.vector.tensor_scalar_mul(out=mb, in0=mb, scalar1=-1.0)
                    for wq in range(gw):
                        nc.scalar.activation(
                            out=X_t[:, wq * 128 : (wq + 1) * 128],
                            in_=X_t[:, wq * 128 : (wq + 1) * 128],
                            func=AF.Identity,
                            scale=rstd_t[:, wq : wq + 1],
                            bias=mb_t[:, wq : wq + 1],
                        )
                    nc.vector.tensor_tensor(out=_r3(Y, 128), in0=_r3(X, 128), in1=_bcast_mid(g1r, gw), op=AX.mult)
                    nc.vector.tensor_tensor(out=_r3(X, 128), in0=_r3(Y, 128), in1=_bcast_mid(b1r, gw), op=AX.add)
                    nc.scalar.activation(out=hB[:, sl], in_=X, func=AF.Relu)
                # xws1 = dis * (hB @ conv_w.T) for this chunk's windows
                for w in range(w0c, w1c):
                    tp = psA.tile([128, 128], F32, tag="ps128")
                    nc.tensor.transpose(tp[:], hB[:, ws(w)], identf[:])
                    ht = hpool.tile([128, 128], BF, tag="ht")
                    nc.scalar.copy(out=ht[:], in_=tp[:])
                    xp = psA.tile([128, 128], F32, tag="ps128")
                    nc.tensor.matmul(xp[:], lhsT=ht[:], rhs=cw[:], start=True, stop=True)
                    nc.scalar.activation(
                        out=xws1[:, ws(w)], in_=xp[:], func=AF.Copy, scale=dic[:, w : w + 1]
                    )
                csz = CHUNK_SZ[c]
                lx = dpool.tile([csz, H], BF, tag=f"lx{c}", name=f"lx{c}")
                nc.sync.dma_start(
                    out=lx[:].rearrange("(w p) f -> p w f", p=128),
                    in_=_r3(xws1[:, w0c * 128 : w1c * 128], 128),
                )
                sup = SUP_OF_CHUNK[c]
                dst = ctx["tb"][sup][:][TB_OFF[c] : TB_OFF[c] + 8 * csz, :]
                nc.gpsimd.collective_compute(
                    "AllGather",
                    AX.bypass,
                    replica_groups=[list(range(NCORES))],
                    ins=[lx.opt()],
                    outs=[dst],
                )

            def call_tile(s_step, s, g, h):
                ctx = ctxs[s_step]
                key = (s, g, h)
                if key not in ctx["call_tiles"]:
                    h0_, h1_, off = halves[(s, g)][h]
                    nb = h1_ - h0_
                    if nb == 0:
                        ctx["call_tiles"][key] = None
                    else:
                        gt = gpool.tile([128, gmax * H], BF, tag="gath")
                        nc.gpsimd.dma_gather(
                            gt[:, : nb * H].rearrange("p (b e) -> p b e", e=H),
                            ctx["tb"][s][:],
                            idx_sb[s][:, off : off + nb * 8],
                            nb * 128,
                            nb * 128,
                            H,
                            single_packet=False,
                            queue_num=qrr[0],
                        )
                        qrr[0] = (qrr[0] + 1) % 4
                        ctx["call_tiles"][key] = gt
                return ctx["call_tiles"][key]

            def prefetch(s_step, grp):
                # super-0 leads deeper than super-1 (super-1 tables land later);
                # keep total in-flight <= gpool bufs (12)
                for gg in range(grp, min(grp + 3, NGRP)):
                    call_tile(s_step, 0, gg, 0)
                    call_tile(s_step, 0, gg, 1)
                for gg in range(grp, min(grp + 2, NGRP)):
                    call_tile(s_step, 1, gg, 0)
                    call_tile(s_step, 1, gg, 1)

            def w_tile(s_step, w):
                ctx = ctxs[s_step]
                if w not in ctx["w_tiles"]:
                    wt = wpool.tile([128, wmax * 128], BF, tag="W")
                    offs = []
                    o = 0
                    for s in range(NSUP):
                        nb = int(B[w, s])
                        offs.append(o)
                        if nb > 0:
                            nc.vector.tensor_tensor(
                                out=_r3(wt[:, o * 128 : (o + nb) * 128], 128),
                                in0=tl_sb[s][:, int(pw[w, s]) : int(pw[w + 1, s])].to_broadcast([128, nb, 128]),
                                in1=_bcast_mid(iot[:], nb),
                                op=AX.is_equal,
                            )
                        o += nb
                    ctx["w_tiles"][w] = (wt, offs)
                return ctx["w_tiles"][w]

            def emit_groups(s_step, glo, ghi):
                ctx = ctxs[s_step]
                state = hB if s_step == 1 else h0
                xws_s = xws1 if s_step == 1 else xws0
                hdst = hA
                for grp in range(glo, ghi):
                    bg = grp * 4
                    prefetch(s_step, grp)
                    gw = min(4, NW - bg)
                    pm = psM.tile([128, 4 * 128], F32, tag="msg")
                    for wq in range(gw):
                        w = bg + wq
                        dst = pm[:, wq * 128 : (wq + 1) * 128]
                        nc.tensor.matmul(dst, lhsT=identb[:], rhs=xws_s[:, ws(w)], start=True, stop=False)
                        nblk = int(B[w].sum())
                        bi = 0
                        for s in range(NSUP):
                            for _ in range(int(B[w, s])):
                                gidx = ctx["blkpos"][s]
                                hh = halves[(s, grp)]
                                h = 0 if gidx < hh[0][1] else 1
                                h0_, h1_, _off = hh[h]
                                ct = call_tile(s_step, s, grp, h)
                                loc = gidx - h0_
                                wt_, woffs = w_tile(s_step, w)
                                wloc = woffs[s] + (gidx - int(pw[w, s]))
                                nc.tensor.matmul(
                                    dst,
                                    lhsT=wt_[:, wloc * 128 : (wloc + 1) * 128],
                                    rhs=ct[:].rearrange("p (b e) -> p b e", e=H)[:, loc, :],
                                    start=False,
                                    stop=(bi == nblk - 1),
                                )
                                ctx["blkpos"][s] += 1
                                bi += 1
                    sl = slice(bg * 128, (bg + gw) * 128)
                    E1_t = tpool.tile([128, 4 * 128], F32, tag="ep_E1")
                    E1 = E1_t[:, : gw * 128]
                    E2_t = tpool.tile([128, 4 * 128], F32, tag="ep_E2")
                    E2 = E2_t[:, : gw * 128]
                    E3_t = tpool.tile([128, 4 * 128], F32, tag="ep_E3")
                    E3 = E3_t[:, : gw * 128]
                    nc.vector.tensor_tensor(
                        out=_r3(E1, 128), in0=_r3(state[:, sl], 128), in1=_bcast_mid(rootr, gw), op=AX.add
                    )
                    for wq in range(gw):
                        w = bg + wq
                        nc.scalar.activation(
                            out=E2_t[:, wq * 128 : (wq + 1) * 128],
                            in_=E1_t[:, wq * 128 : (wq + 1) * 128],
                            func=AF.Relu,
                            scale=dvc[:, w : w + 1],
                        )
                        nc.scalar.activation(
                            out=E3_t[:, wq * 128 : (wq + 1) * 128],
                            in_=pm[:, wq * 128 : (wq + 1) * 128],
                            func=AF.Copy,
                            scale=dic[:, w : w + 1],
                        )
                    nc.vector.tensor_tensor(out=E2, in0=E3, in1=E2, op=AX.add)
                    if s_step == 0:
                        # rank-1 lin-bias message term: + q_t * b2
                        for wq in range(gw):
                            w = bg + wq
                            nc.scalar.activation(
                                out=E3_t[:, wq * 128 : (wq + 1) * 128],
                                in_=b2r,
                                func=AF.Copy,
                                scale=qcl[:, w : w + 1],
                            )
                        nc.vector.tensor_tensor(out=E2, in0=E3, in1=E2, op=AX.add)
                    nc.vector.tensor_tensor(
                        out=_r3(hdst[:, sl], 128), in0=_r3(E2, 128), in1=_bcast_mid(convbr, gw), op=AX.add
                    )

            # software-pipelined emission: step-1 publishes overlap step-0 consumption
            emit_groups(0, 0, 3)
            emit_publish(0)
            emit_groups(0, 3, 6)
            emit_publish(1)
            emit_groups(0, 6, 9)
            emit_publish(2)
            emit_groups(0, 9, NGRP)
            emit_publish(3)
            emit_groups(1, 0, NGRP)

            # ---- output ----
            nc.sync.dma_start(
                out=out_ext[:].rearrange("(w p) f -> p w f", p=128),
                in_=_r3(hA[:], 128),
            )
    nc.compile()
    return nc


def _rep(v):
    return np.tile(np.asarray(v, np.float32).reshape(1, H), (128, 1))


def kernel_with_results(**inputs):
    in_feat = np.asarray(inputs["in_feat"], np.float32)
    row = np.asarray(inputs["row"]).astype(np.int64)
    col = np.asarray(inputs["col"]).astype(np.int64)
    lin_w = np.asarray(inputs["lin_w"], np.float32)
    lin_b = np.asarray(inputs["lin_b"], np.float32)
    conv_w = np.asarray(inputs["conv_w"], np.float32)
    conv_b = np.asarray(inputs["conv_b"], np.float32)
    root_emb = np.asarray(inputs["root_emb"], np.float32)
    ln_gamma = np.asarray(inputs["ln_gamma"], np.float32)
    ln_beta = np.asarray(inputs["ln_beta"], np.float32)

    g = _prep_graph(row, col)
    nc = _build(g["B"], g["gpos"], g["NBLK"])

    ift_t = in_feat.T  # [IN, N]
    ift_full = np.zeros((IN, NCORES * PADN), BF16)
    for k in range(NCORES):
        ift_full[:, k * PADN : k * PADN + NPC] = ift_t[:, k * NPC : (k + 1) * NPC].astype(BF16)
    # fused step-0 weights: xw0 = X @ W2 + b2, W2 = (conv_w @ lin_w).T
    w2 = (conv_w.astype(np.float64) @ lin_w.astype(np.float64)).astype(np.float32)
    b2 = (conv_w.astype(np.float64) @ lin_b.astype(np.float64)).astype(np.float32)
    consts = np.concatenate(
        [_rep(lin_b), _rep(root_emb[0]), _rep(conv_b), _rep(ln_gamma[1]), _rep(ln_beta[1]), _rep(b2)],
        axis=1,
    )
    iota = np.tile(np.arange(128, dtype=np.float32), (128, 1)).astype(BF16)
    lin_wT = np.ascontiguousarray(lin_w.T).astype(BF16)
    w2T = np.ascontiguousarray(w2.T).astype(BF16)
    conv_wT = np.ascontiguousarray(conv_w.T).astype(BF16)

    in_maps = []
    for k in range(NCORES):
        ift_k = np.zeros((IN, PADN), BF16)
        ift_k[:, :NPC] = ift_t[:, k * NPC : (k + 1) * NPC].astype(BF16)
        m = {
            "ift": ift_full,
            "iftl": ift_k,
            "lin_wT": lin_wT,
            "w2T": w2T,
            "conv_wT": conv_wT,
            "consts": consts,
            "iota": iota,
            "discols": g["dis_cols"][k],
            "disfull": g["dis_full"],
            "dinvcols": g["dinv_cols"][k],
            "qcols": g["q_cols"][k],
        }
        for s in range(NSUP):
            m[f"idx{s}"] = g["idx_w"][s][k]
            m[f"tloc{s}"] = np.ascontiguousarray(g["tlocs"][s][k])
        in_maps.append(m)

    res = run_bass_kernel_spmd(nc, in_maps, list(range(NCORES)))
    out = np.concatenate(
        [np.asarray(res.results[k]["out"])[:NPC] for k in range(NCORES)], axis=0
    )
    return out.astype(np.float32), res


def kernel(**inputs):
    out, _ = kernel_with_results(**inputs)
    return out


# revision 21
# speedup vs baseline: 1.0086x; 1.0086x over previous
"""EnhancedGCN on 8 Trainium2 NeuronCores (Bass/Tile, SPMD).

Strategy: 1D node partition (6250 nodes/core, padded to 6272). Step-0's
gather table xws0 = dis * (in_feat @ W2) (W2 = conv_w @ lin_w pre-fused
host-side) is computed REPLICATED on every core straight into local DRAM —
no step-0 collective at all. The lin bias term is rank-1 and folds into the
epilogue through a host-precomputed per-node scalar q = dis_t * sum_dis.
Step-1 publishes xws1 = dis * (hB @ conv_w.T) via 4 chunked AllGathers
(Shared outputs, triggered from the Sync engine so they never queue behind
gather calls). Per-edge messages are fetched with dma_gather (4 SWDGE
queues, round-robin assignment, super-0-leading prefetch) and reduced into
per-target sums with 0/1 selection-matrix matmuls accumulating in PSUM
(self-loops enter as an identity-matmul block). Edge weights
ew = dis[t]*dis[s] are separable: dis[s] pre-scales the table, dis[t]
post-scales the message sum. Host-side work is limited to graph-structure
prep (sorting edges into target windows, block padding, int16 index
streams) and weight transposes/fusion.
"""
import sys

sys.path.insert(0, "/opt/trn_rl_repo")

import numpy as np
import ml_dtypes

import concourse.bass as bass
import concourse.bacc as bacc
import concourse.tile as tile
import concourse.mybir as mybir
from concourse.bass_utils import run_bass_kernel_spmd
from concourse.masks import make_identity

BF16 = ml_dtypes.bfloat16
N, IN, H = 50000, 256, 128
NCORES = 8
NPC = N // NCORES  # 6250
NW = (NPC + 127) // 128  # 49
PADN = NW * 128  # 6272
LN_EPS = 1e-5
NGRP = (NW + 3) // 4  # 13 groups of 4 windows

# table chunks (windows per chunk) and the two gather super-streams
CHUNK_W = [12, 12, 12, 13]
CHUNK_W0 = [0, 12, 24, 36, 49]
NCHUNK = 4
NSUP = 2  # chunks 0+1 -> super 0 (windows 0..23), chunks 2+3 -> super 1
SUP_OF_CHUNK = [0, 0, 1, 1]
# table row layout per super: [chunkA: 8 ranks x szA | chunkB: 8 ranks x szB]
CHUNK_SZ = [cw * 128 for cw in CHUNK_W]
TBL_ROWS = [8 * (CHUNK_SZ[0] + CHUNK_SZ[1]), 8 * (CHUNK_SZ[2] + CHUNK_SZ[3])]
TB_OFF = [0, 8 * CHUNK_SZ[0], 0, 8 * CHUNK_SZ[2]]

F32 = mybir.dt.float32
BF = mybir.dt.bfloat16
I16 = mybir.dt.int16
AX = mybir.AluOpType
AF = mybir.ActivationFunctionType


def _bcast_mid(ap, n):
    """[128, F] AP -> [128, n, F] with stride-0 middle dim."""
    a = ap.copy()
    a.ap = [a.ap[0], [0, n]] + a.ap[1:]
    return a


def _r3(ap, f):
    return ap.rearrange("p (w f) -> p w f", f=f)


def _wrap_idx(idx):
    """flat idx [n] (n % 16 == 0) -> [128, n/16] int16 wrapped + replicated."""
    n = len(idx)
    t = idx.reshape(n // 16, 16).T.astype(np.int16)
    return np.tile(t, (8, 1))


def _prep_graph(row, col):
    """Graph-structure-only preprocessing (row/col ints)."""
    deg = np.bincount(row, minlength=N).astype(np.float64) + 1.0
    dis_f = 1.0 / np.sqrt(deg)
    dinv_f = 1.0 / deg
    # q_t = dis_t * (sum_{s in N(t)} dis_s + dis_t): rank-1 lin-bias epilogue
    csum = np.bincount(row, weights=dis_f[col], minlength=N) + dis_f
    q_f = dis_f * csum

    core = row // NPC
    src_core = col // NPC
    src_off = col % NPC
    src_w = src_off >> 7
    src_chunk = np.digitize(src_w, CHUNK_W0[1:4])  # 0..3
    src_sup = (src_chunk >= 2).astype(np.int64)
    base = np.asarray(TB_OFF)[src_chunk]
    csz = np.asarray(CHUNK_SZ)[src_chunk]
    cwn = np.asarray(CHUNK_W)[src_chunk]
    w0 = np.asarray(CHUNK_W0)[src_chunk]
    # table rows within a (chunk, core) block are partition-major
    # (row = p*cw + w_local) so the SBUF->DRAM table write is cw contiguous
    # rows per partition instead of one 256B descriptor per row
    src_p = src_off & 127
    src_wl = src_w - w0
    src_idx = base + src_core * csz + src_p * cwn + src_wl

    per_core = []
    counts = np.zeros((NCORES, NW, NSUP), np.int64)
    for k in range(NCORES):
        m = core == k
        tgt = (row[m] - k * NPC).astype(np.int64)
        sidx = src_idx[m]
        ssup = src_sup[m]
        w = tgt >> 7
        order = np.argsort(w, kind="stable")
        tgt, sidx, ssup, w = tgt[order], sidx[order], ssup[order], w[order]
        ents = []
        bounds = np.searchsorted(w, np.arange(NW + 1))
        for wi in range(NW):
            sl = slice(bounds[wi], bounds[wi + 1])
            s_w, t_w, u_w = sidx[sl], tgt[sl] - (wi << 7), ssup[sl]
            by_sup = []
            for s in range(NSUP):
                mm = u_w == s
                by_sup.append((s_w[mm], t_w[mm]))
                counts[k, wi, s] = int(mm.sum())
            ents.append(by_sup)
        per_core.append(ents)

    B = np.ceil(counts.max(axis=0) / 128).astype(np.int64)  # [NW, NSUP]
    NBLK = B.sum(axis=0).astype(np.int64)  # per super
    gpos = np.zeros((NSUP, NGRP + 1), np.int64)
    for s in range(NSUP):
        pref = np.concatenate([[0], np.cumsum(B[:, s])])
        for g in range(NGRP + 1):
            gpos[s, g] = pref[min(g * 4, NW)]

    rng = np.random.default_rng(12345)
    idx_streams = [np.empty((NCORES, int(NBLK[s]) * 128), np.int64) for s in range(NSUP)]
    for s in range(NSUP):
        idx_streams[s][:] = rng.integers(0, TBL_ROWS[s], idx_streams[s].shape)
    tlocs = [np.full((NCORES, 128, int(NBLK[s])), -1.0, np.float32) for s in range(NSUP)]

    for k in range(NCORES):
        pos = [0] * NSUP
        for wi in range(NW):
            for s in range(NSUP):
                s_w, t_w = per_core[k][wi][s]
                n = len(s_w)
                p = pos[s]
                idx_streams[s][k, p * 128 : p * 128 + n] = s_w
                j = np.arange(n)
                tlocs[s][k, j % 128, p + j // 128] = t_w
                pos[s] += int(B[wi, s])

    # wrap idx per half-call segment (aligned to group boundaries, split in two)
    idx_w = [None] * NSUP
    for s in range(NSUP):
        per_core_w = [[] for _ in range(NCORES)]
        for g in range(NGRP):
            b0, b1 = int(gpos[s, g]), int(gpos[s, g + 1])
            mid = b0 + (b1 - b0 + 1) // 2
            for (h0_, h1_) in ((b0, mid), (mid, b1)):
                if h1_ > h0_:
                    for k in range(NCORES):
                        per_core_w[k].append(
                            _wrap_idx(idx_streams[s][k, h0_ * 128 : h1_ * 128])
                        )
        idx_w[s] = np.stack([np.concatenate(x, axis=1) for x in per_core_w])

    dis_cols = np.zeros((NCORES, 128, NW), np.float32)
    dinv_cols = np.ones((NCORES, 128, NW), np.float32)
    q_cols = np.zeros((NCORES, 128, NW), np.float32)
    dis_full = np.zeros((128, NCORES * NW), np.float32)
    for k in range(NCORES):
        v = np.zeros(PADN, np.float64)
        v[:NPC] = dis_f[k * NPC : (k + 1) * NPC]
        dis_cols[k] = v.reshape(NW, 128).T
        dis_full[:, k * NW : (k + 1) * NW] = dis_cols[k]
        u = np.ones(PADN, np.float64)
        u[:NPC] = dinv_f[k * NPC : (k + 1) * NPC]
        dinv_cols[k] = u.reshape(NW, 128).T
        qv = np.zeros(PADN, np.float64)
        qv[:NPC] = q_f[k * NPC : (k + 1) * NPC]
        q_cols[k] = qv.reshape(NW, 128).T

    return dict(
        B=B,
        NBLK=NBLK,
        gpos=gpos,
        idx_w=idx_w,
        tlocs=[t.astype(BF16) for t in tlocs],
        dis_cols=dis_cols,
        dinv_cols=dinv_cols,
        q_cols=q_cols,
        dis_full=dis_full,
    )


def _build(B, gpos, NBLK):
    nc = bacc.Bacc("TRN2", target_bir_lowering=False, debug=False, num_swdge_queues=4)

    ift = nc.dram_tensor("ift", [IN, NCORES * PADN], BF, kind="ExternalInput")
    iftl = nc.dram_tensor("iftl", [IN, PADN], BF, kind="ExternalInput")
    lin_wT = nc.dram_tensor("lin_wT", [IN, H], BF, kind="ExternalInput")
    w2T = nc.dram_tensor("w2T", [IN, H], BF, kind="ExternalInput")
    conv_wT = nc.dram_tensor("conv_wT", [H, H], BF, kind="ExternalInput")
    consts = nc.dram_tensor("consts", [128, 6 * H], F32, kind="ExternalInput")
    iota_in = nc.dram_tensor("iota", [128, 128], BF, kind="ExternalInput")
    discols = nc.dram_tensor("discols", [128, NW], F32, kind="ExternalInput")
    disfull = nc.dram_tensor("disfull", [128, NCORES * NW], F32, kind="ExternalInput")
    dinvcols = nc.dram_tensor("dinvcols", [128, NW], F32, kind="ExternalInput")
    qcols = nc.dram_tensor("qcols", [128, NW], F32, kind="ExternalInput")
    idx_t = [
        nc.dram_tensor(f"idx{s}", [128, int(NBLK[s]) * 8], I16, kind="ExternalInput")
        for s in range(NSUP)
    ]
    tloc_t = [
        nc.dram_tensor(f"tloc{s}", [128, int(NBLK[s])], BF, kind="ExternalInput")
        for s in range(NSUP)
    ]
    # partition-major output: out[p, w*H+f] = h[w*128+p, f]; host reassembles
    out_ext = nc.dram_tensor("out", [128, PADN], F32, kind="ExternalOutput")

    def ws(w):
        return slice(w * 128, (w + 1) * 128)

    # per-(stream, group) half-call boundaries + column offset into wrapped idx
    halves = {}
    for s in range(NSUP):
        off = 0
        for g in range(NGRP):
            b0, b1 = int(gpos[s, g]), int(gpos[s, g + 1])
            mid = b0 + (b1 - b0 + 1) // 2
            hs = []
            for (h0_, h1_) in ((b0, mid), (mid, b1)):
                hs.append((h0_, h1_, off))
                off += (h1_ - h0_) * 8
            halves[(s, g)] = hs
    gmax = max(h1 - h0 for v in halves.values() for (h0, h1, _) in v)
    wmax = int(B.sum(axis=1).max())
    pw = np.concatenate([np.zeros((1, NSUP), np.int64), np.cumsum(B, axis=0)], axis=0)

    with tile.TileContext(nc) as tc:
        with (
            tc.tile_pool(name="const", bufs=1) as cpool,
            tc.tile_pool(name="state", bufs=1) as spool,
            tc.tile_pool(name="iftp", bufs=2) as ipool,
            tc.tile_pool(name="lftp", bufs=4) as lpool,
            tc.tile_pool(name="ht", bufs=4) as hpool,
            tc.tile_pool(name="gath", bufs=12) as gpool,
            tc.tile_pool(name="wp", bufs=5) as wpool,
            tc.tile_pool(name="tmp", bufs=1) as tpool,
            tc.tile_pool(name="psA", bufs=4, space="PSUM") as psA,
            tc.tile_pool(name="psM", bufs=4, space="PSUM") as psM,
            tc.tile_pool(name="dram", bufs=1, space="DRAM") as dpool,
        ):
            identf = cpool.tile([128, 128], F32)
            make_identity(nc, identf[:])
            identb = cpool.tile([128, 128], BF)
            nc.vector.tensor_copy(out=identb[:], in_=identf[:])
            cst = cpool.tile([128, 6 * H], F32)
            nc.sync.dma_start(out=cst[:], in_=consts[:])
            linb, rootr, convbr, g1r, b1r, b2r = (
                cst[:, i * H : (i + 1) * H] for i in range(6)
            )
            iot = cpool.tile([128, 128], BF)
            nc.sync.dma_start(out=iot[:], in_=iota_in[:])
            cw = cpool.tile([128, H], BF)
            nc.sync.dma_start(out=cw[:], in_=conv_wT[:])
            lw0 = cpool.tile([128, H], BF)
            nc.sync.dma_start(out=lw0[:], in_=lin_wT[0:128, :])
            lw1 = cpool.tile([128, H], BF)
            nc.sync.dma_start(out=lw1[:], in_=lin_wT[128:256, :])
            w20 = cpool.tile([128, H], BF)
            nc.sync.dma_start(out=w20[:], in_=w2T[0:128, :])
            w21 = cpool.tile([128, H], BF)
            nc.sync.dma_start(out=w21[:], in_=w2T[128:256, :])
            dic = cpool.tile([128, NW], F32)
            nc.sync.dma_start(out=dic[:], in_=discols[:])
            dfl = cpool.tile([128, NCORES * NW], F32)
            nc.sync.dma_start(out=dfl[:], in_=disfull[:])
            dvc = cpool.tile([128, NW], F32)
            nc.sync.dma_start(out=dvc[:], in_=dinvcols[:])
            qcl = cpool.tile([128, NW], F32)
            nc.sync.dma_start(out=qcl[:], in_=qcols[:])
            tl_sb = []
            idx_sb = []
            for s in range(NSUP):
                t = cpool.tile([128, int(NBLK[s])], BF, name=f"tl{s}")
                nc.sync.dma_start(out=t[:], in_=tloc_t[s][:])
                tl_sb.append(t)
                t2 = cpool.tile([128, int(NBLK[s]) * 8], I16, name=f"ix{s}")
                nc.sync.dma_start(out=t2[:], in_=idx_t[s][:])
                idx_sb.append(t2)

            h0 = spool.tile([128, PADN], F32, tag="h0")
            hA = spool.tile([128, PADN], F32, tag="hA")
            hB = spool.tile([128, PADN], F32, tag="hB")
            xws0 = spool.tile([128, PADN], BF, tag="xws0")
            xws1 = spool.tile([128, PADN], BF, tag="xws1")

            # ---- step-0 table: replicated full-graph xws0 into local DRAM ----
            tb0 = [
                dpool.tile([TBL_ROWS[0], H], BF, tag="tb0_0", name="tb0A"),
                dpool.tile([TBL_ROWS[1], H], BF, tag="tb0_1", name="tb0B"),
            ]
            for c in range(NCHUNK):
                w0c, w1c = CHUNK_W0[c], CHUNK_W0[c + 1]
                cwn = w1c - w0c
                csz = CHUNK_SZ[c]
                sup = SUP_OF_CHUNK[c]
                for k in range(NCORES):
                    col0 = k * PADN + w0c * 128
                    i0 = ipool.tile([128, CHUNK_SZ[3]], BF, tag="ifta")
                    nc.sync.dma_start(out=i0[:, :csz], in_=ift[0:128, col0 : col0 + csz])
                    i1 = ipool.tile([128, CHUNK_SZ[3]], BF, tag="iftb")
                    nc.sync.dma_start(out=i1[:, :csz], in_=ift[128:256, col0 : col0 + csz])
                    for wq in range(cwn):
                        xp = psA.tile([128, 128], F32, tag="ps128")
                        nc.tensor.matmul(
                            xp[:], lhsT=i0[:, ws(wq)], rhs=w20[:], start=True, stop=False
                        )
                        nc.tensor.matmul(
                            xp[:], lhsT=i1[:, ws(wq)], rhs=w21[:], start=False, stop=True
                        )
                        # i1's window is dead after its matmul: reuse it as the
                        # bf16 staging for the table write
                        nc.scalar.activation(
                            out=i1[:, ws(wq)],
                            in_=xp[:],
                            func=AF.Copy,
                            scale=dfl[:, k * NW + w0c + wq : k * NW + w0c + wq + 1],
                        )
                    nc.sync.dma_start(
                        out=tb0[sup][:][
                            TB_OFF[c] + k * csz : TB_OFF[c] + (k + 1) * csz, :
                        ].rearrange("(p w) f -> p w f", w=cwn),
                        in_=_r3(i1[:, :csz], 128),
                    )

            # ---- local shard: h0 (lin) and xws0 (for self-loop matmuls) ----
            for w in range(NW):
                i0 = lpool.tile([128, 128], BF, tag="lfta")
                nc.sync.dma_start(out=i0[:], in_=iftl[0:128, ws(w)])
                i1 = lpool.tile([128, 128], BF, tag="lftb")
                nc.sync.dma_start(out=i1[:], in_=iftl[128:256, ws(w)])
                hp = psA.tile([128, 128], F32, tag="ps128")
                nc.tensor.matmul(hp[:], lhsT=i0[:], rhs=lw0[:], start=True, stop=False)
                nc.tensor.matmul(hp[:], lhsT=i1[:], rhs=lw1[:], start=False, stop=True)
                nc.vector.tensor_tensor(out=h0[:, ws(w)], in0=hp[:], in1=linb, op=AX.add)
                xp = psA.tile([128, 128], F32, tag="ps128")
                nc.tensor.matmul(xp[:], lhsT=i0[:], rhs=w20[:], start=True, stop=False)
                nc.tensor.matmul(xp[:], lhsT=i1[:], rhs=w21[:], start=False, stop=True)
                nc.scalar.activation(
                    out=xws0[:, ws(w)], in_=xp[:], func=AF.Copy, scale=dic[:, w : w + 1]
                )

            ctxs = {
                0: dict(tb=tb0, call_tiles={}, w_tiles={}, blkpos=[0] * NSUP),
            }
            qrr = [0]  # round-robin SWDGE queue assignment

            def emit_publish(c):
                """Step-1: LN + xws1 for chunk c + publish + AllGather (Shared out)."""
                if 1 not in ctxs:
                    tbA = dpool.tile([TBL_ROWS[0], H], BF, tag="tb1_0", name="tb1A")
                    tbB = dpool.tile([TBL_ROWS[1], H], BF, tag="tb1_1", name="tb1B")
                    ctxs[1] = dict(tb=[tbA, tbB], call_tiles={}, w_tiles={}, blkpos=[0] * NSUP)
                ctx = ctxs[1]
                w0c, w1c = CHUNK_W0[c], CHUNK_W0[c + 1]
                # residual + layernorm + relu -> hB for this chunk's windows
                for g in range(w0c, w1c, 4):
                    gw = min(4, w1c - g)
                    sl = slice(g * 128, (g + gw) * 128)
                    X_t = tpool.tile([128, 4 * 128], F32, tag="ln_X")
                    X = X_t[:, : gw * 128]
                    Y_t = tpool.tile([128, 4 * 128], F32, tag="ln_Y")
                    Y = Y_t[:, : gw * 128]
                    nc.vector.tensor_tensor(out=X, in0=hA[:, sl], in1=h0[:, sl], op=AX.add)
                    mu_t = tpool.tile([128, 4], F32, tag="ln_mu")
                    mu = mu_t[:, :gw]
                    nc.vector.tensor_reduce(out=mu, in_=_r3(X, 128), axis=mybir.AxisListType.X, op=AX.add)
                    nc.vector.tensor_scalar_mul(out=mu, in0=mu, scalar1=1.0 / 128.0)
                    nc.vector.tensor_tensor(out=Y, in0=X, in1=X, op=AX.mult)
                    var_t = tpool.tile([128, 4], F32, tag="ln_var")
                    var = var_t[:, :gw]
                    nc.vector.tensor_reduce(out=var, in_=_r3(Y, 128), axis=mybir.AxisListType.X, op=AX.add)
                    mm_t = tpool.tile([128, 4], F32, tag="ln_mm")
                    mm = mm_t[:, :gw]
                    nc.vector.tensor_tensor(out=mm, in0=mu, in1=mu, op=AX.mult)
                    nc.vector.tensor_scalar(
                        out=var, in0=var, scalar1=1.0 / 128.0, scalar2=LN_EPS, op0=AX.mult, op1=AX.add
                    )
                    nc.vector.tensor_tensor(out=var, in0=var, in1=mm, op=AX.subtract)
                    sd_t = tpool.tile([128, 4], F32, tag="ln_sd")
                    sd = sd_t[:, :gw]
                    nc.scalar.activation(out=sd, in_=var, func=AF.Sqrt)
                    rstd_t = tpool.tile([128, 4], F32, tag="ln_rs")
                    rstd = rstd_t[:, :gw]
                    nc.vector.reciprocal(out=rstd, in_=sd)
                    mb_t = tpool.tile([128, 4], F32, tag="ln_mb")
                    mb = mb_t[:, :gw]
                    nc.vector.tensor_tensor(out=mb, in0=mu, in1=rstd, op=AX.mult)
                    nc.vector.tensor_scalar_mul(out=mb, in0=mb, scalar1=-1.0)
                    for wq in range(gw):
                        nc.scalar.activation(
                            out=X_t[:, wq * 128 : (wq + 1) * 128],
                            in_=X_t[:, wq * 128 : (wq + 1) * 128],
                            func=AF.Identity,
                            scale=rstd_t[:, wq : wq + 1],
                            bias=mb_t[:, wq : wq + 1],
                        )
                    nc.vector.tensor_tensor(out=_r3(Y, 128), in0=_r3(X, 128), in1=_bcast_mid(g1r, gw), op=AX.mult)
                    nc.vector.tensor_tensor(out=_r3(X, 128), in0=_r3(Y, 128), in1=_bcast_mid(b1r, gw), op=AX.add)
                    nc.scalar.activation(out=hB[:, sl], in_=X, func=AF.Relu)
                # xws1 = dis * (hB @ conv_w.T) for this chunk's windows
                for w in range(w0c, w1c):
                    tp = psA.tile([128, 128], F32, tag="ps128")
                    nc.tensor.transpose(tp[:], hB[:, ws(w)], identf[:])
                    ht = hpool.tile([128, 128], BF, tag="ht")
                    nc.scalar.copy(out=ht[:], in_=tp[:])
                    xp = psA.tile([128, 128], F32, tag="ps128")
                    nc.tensor.matmul(xp[:], lhsT=ht[:], rhs=cw[:], start=True, stop=True)
                    nc.scalar.activation(
                        out=xws1[:, ws(w)], in_=xp[:], func=AF.Copy, scale=dic[:, w : w + 1]
                    )
                csz = CHUNK_SZ[c]
                lx = dpool.tile([csz, H], BF, tag=f"lx{c}", name=f"lx{c}")
                nc.sync.dma_start(
                    out=lx[:].rearrange("(p w) f -> p w f", w=w1c - w0c),
                    in_=_r3(xws1[:, w0c * 128 : w1c * 128], 128),
                )
                sup = SUP_OF_CHUNK[c]
                dst = ctx["tb"][sup][:][TB_OFF[c] : TB_OFF[c] + 8 * csz, :]
                nc.gpsimd.collective_compute(
                    "AllGather",
                    AX.bypass,
                    replica_groups=[list(range(NCORES))],
                    ins=[lx.opt()],
                    outs=[dst],
                )

            def call_tile(s_step, s, g, h):
                ctx = ctxs[s_step]
                key = (s, g, h)
                if key not in ctx["call_tiles"]:
                    h0_, h1_, off = halves[(s, g)][h]
                    nb = h1_ - h0_
                    if nb == 0:
                        ctx["call_tiles"][key] = None
                    else:
                        gt = gpool.tile([128, gmax * H], BF, tag="gath")
                        nc.gpsimd.dma_gather(
                            gt[:, : nb * H].rearrange("p (b e) -> p b e", e=H),
                            ctx["tb"][s][:],
                            idx_sb[s][:, off : off + nb * 8],
                            nb * 128,
                            nb * 128,
                            H,
                            single_packet=False,
                            queue_num=qrr[0],
                        )
                        qrr[0] = (qrr[0] + 1) % 4
                        ctx["call_tiles"][key] = gt
                return ctx["call_tiles"][key]

            def prefetch(s_step, grp):
                # super-0 leads deeper than super-1 (super-1 tables land later);
                # keep total in-flight <= gpool bufs (12)
                for gg in range(grp, min(grp + 3, NGRP)):
                    call_tile(s_step, 0, gg, 0)
                    call_tile(s_step, 0, gg, 1)
                for gg in range(grp, min(grp + 2, NGRP)):
                    call_tile(s_step, 1, gg, 0)
                    call_tile(s_step, 1, gg, 1)

            def w_tile(s_step, w):
                ctx = ctxs[s_step]
                if w not in ctx["w_tiles"]:
                    wt = wpool.tile([128, wmax * 128], BF, tag="W")
                    offs = []
                    o = 0
                    for s in range(NSUP):
                        nb = int(B[w, s])
                        offs.append(o)
                        if nb > 0:
                            nc.vector.tensor_tensor(
                                out=_r3(wt[:, o * 128 : (o + nb) * 128], 128),
                                in0=tl_sb[s][:, int(pw[w, s]) : int(pw[w + 1, s])].to_broadcast([128, nb, 128]),
                                in1=_bcast_mid(iot[:], nb),
                                op=AX.is_equal,
                            )
                        o += nb
                    ctx["w_tiles"][w] = (wt, offs)
                return ctx["w_tiles"][w]

            def emit_groups(s_step, glo, ghi):
                ctx = ctxs[s_step]
                state = hB if s_step == 1 else h0
                xws_s = xws1 if s_step == 1 else xws0
                hdst = hA
                for grp in range(glo, ghi):
                    bg = grp * 4
                    prefetch(s_step, grp)
                    gw = min(4, NW - bg)
                    pm = psM.tile([128, 4 * 128], F32, tag="msg")
                    for wq in range(gw):
                        w = bg + wq
                        dst = pm[:, wq * 128 : (wq + 1) * 128]
                        nc.tensor.matmul(dst, lhsT=identb[:], rhs=xws_s[:, ws(w)], start=True, stop=False)
                        nblk = int(B[w].sum())
                        bi = 0
                        for s in range(NSUP):
                            for _ in range(int(B[w, s])):
                                gidx = ctx["blkpos"][s]
                                hh = halves[(s, grp)]
                                h = 0 if gidx < hh[0][1] else 1
                                h0_, h1_, _off = hh[h]
                                ct = call_tile(s_step, s, grp, h)
                                loc = gidx - h0_
                                wt_, woffs = w_tile(s_step, w)
                                wloc = woffs[s] + (gidx - int(pw[w, s]))
                                nc.tensor.matmul(
                                    dst,
                                    lhsT=wt_[:, wloc * 128 : (wloc + 1) * 128],
                                    rhs=ct[:].rearrange("p (b e) -> p b e", e=H)[:, loc, :],
                                    start=False,
                                    stop=(bi == nblk - 1),
                                )
                                ctx["blkpos"][s] += 1
                                bi += 1
                    sl = slice(bg * 128, (bg + gw) * 128)
                    E1_t = tpool.tile([128, 4 * 128], F32, tag="ep_E1")
                    E1 = E1_t[:, : gw * 128]
                    E2_t = tpool.tile([128, 4 * 128], F32, tag="ep_E2")
                    E2 = E2_t[:, : gw * 128]
                    E3_t = tpool.tile([128, 4 * 128], F32, tag="ep_E3")
                    E3 = E3_t[:, : gw * 128]
                    nc.vector.tensor_tensor(
                        out=_r3(E1, 128), in0=_r3(state[:, sl], 128), in1=_bcast_mid(rootr, gw), op=AX.add
                    )
                    for wq in range(gw):
                        w = bg + wq
                        nc.scalar.activation(
                            out=E2_t[:, wq * 128 : (wq + 1) * 128],
                            in_=E1_t[:, wq * 128 : (wq + 1) * 128],
                            func=AF.Relu,
                            scale=dvc[:, w : w + 1],
                        )
                        nc.scalar.activation(
                            out=E3_t[:, wq * 128 : (wq + 1) * 128],
                            in_=pm[:, wq * 128 : (wq + 1) * 128],
                            func=AF.Copy,
                            scale=dic[:, w : w + 1],
                        )
                    nc.vector.tensor_tensor(out=E2, in0=E3, in1=E2, op=AX.add)
                    if s_step == 0:
                        # rank-1 lin-bias message term: + q_t * b2
                        for wq in range(gw):
                            w = bg + wq
                            nc.scalar.activation(
                                out=E3_t[:, wq * 128 : (wq + 1) * 128],
                                in_=b2r,
                                func=AF.Copy,
                                scale=qcl[:, w : w + 1],
                            )
                        nc.vector.tensor_tensor(out=E2, in0=E3, in1=E2, op=AX.add)
                    nc.vector.tensor_tensor(
                        out=_r3(hdst[:, sl], 128), in0=_r3(E2, 128), in1=_bcast_mid(convbr, gw), op=AX.add
                    )
                    if s_step == 1:
                        # stream the finished group straight out (contiguous
                        # per-partition rows; H == 128 makes layouts identical)
                        nc.sync.dma_start(out=out_ext[:, sl], in_=hdst[:, sl])

            # software-pipelined emission: step-1 publishes overlap step-0 consumption
            emit_groups(0, 0, 3)
            emit_publish(0)
            emit_groups(0, 3, 6)
            emit_publish(1)
            emit_groups(0, 6, 9)
            emit_publish(2)
            emit_groups(0, 9, NGRP)
            emit_publish(3)
            emit_groups(1, 0, NGRP)
    nc.compile()
    return nc


def _rep(v):
    return np.tile(np.asarray(v, np.float32).reshape(1, H), (128, 1))


def kernel_with_results(**inputs):
    in_feat = np.asarray(inputs["in_feat"], np.float32)
    row = np.asarray(inputs["row"]).astype(np.int64)
    col = np.asarray(inputs["col"]).astype(np.int64)
    lin_w = np.asarray(inputs["lin_w"], np.float32)
    lin_b = np.asarray(inputs["lin_b"], np.float32)
    conv_w = np.asarray(inputs["conv_w"], np.float32)
    conv_b = np.asarray(inputs["conv_b"], np.float32)
    root_emb = np.asarray(inputs["root_emb"], np.float32)
    ln_gamma = np.asarray(inputs["ln_gamma"], np.float32)
    ln_beta = np.asarray(inputs["ln_beta"], np.float32)

    g = _prep_graph(row, col)
    nc = _build(g["B"], g["gpos"], g["NBLK"])

    ift_t = in_feat.T  # [IN, N]
    ift_full = np.zeros((IN, NCORES * PADN), BF16)
    for k in range(NCORES):
        ift_full[:, k * PADN : k * PADN + NPC] = ift_t[:, k * NPC : (k + 1) * NPC].astype(BF16)
    # fused step-0 weights: xw0 = X @ W2 + b2, W2 = (conv_w @ lin_w).T
    w2 = (conv_w.astype(np.float64) @ lin_w.astype(np.float64)).astype(np.float32)
    b2 = (conv_w.astype(np.float64) @ lin_b.astype(np.float64)).astype(np.float32)
    consts = np.concatenate(
        [_rep(lin_b), _rep(root_emb[0]), _rep(conv_b), _rep(ln_gamma[1]), _rep(ln_beta[1]), _rep(b2)],
        axis=1,
    )
    iota = np.tile(np.arange(128, dtype=np.float32), (128, 1)).astype(BF16)
    lin_wT = np.ascontiguousarray(lin_w.T).astype(BF16)
    w2T = np.ascontiguousarray(w2.T).astype(BF16)
    conv_wT = np.ascontiguousarray(conv_w.T).astype(BF16)

    in_maps = []
    for k in range(NCORES):
        ift_k = np.zeros((IN, PADN), BF16)
        ift_k[:, :NPC] = ift_t[:, k * NPC : (k + 1) * NPC].astype(BF16)
        m = {
            "ift": ift_full,
            "iftl": ift_k,
            "lin_wT": lin_wT,
            "w2T": w2T,
            "conv_wT": conv_wT,
            "consts": consts,
            "iota": iota,
            "discols": g["dis_cols"][k],
            "disfull": g["dis_full"],
            "dinvcols": g["dinv_cols"][k],
            "qcols": g["q_cols"][k],
        }
        for s in range(NSUP):
            m[f"idx{s}"] = g["idx_w"][s][k]
            m[f"tloc{s}"] = np.ascontiguousarray(g["tlocs"][s][k])
        in_maps.append(m)

    res = run_bass_kernel_spmd(nc, in_maps, list(range(NCORES)))
    shards = []
    for k in range(NCORES):
        o = np.asarray(res.results[k]["out"])  # [128, NW*H], [p, w*H+f]
        o = o.reshape(128, NW, H).transpose(1, 0, 2).reshape(PADN, H)
        shards.append(o[:NPC])
    out = np.concatenate(shards, axis=0)
    return out.astype(np.float32), res


def kernel(**inputs):
    out, _ = kernel_with_results(**inputs)
    return out


# revision 25
# speedup vs baseline: 1.2058x; 1.1955x over previous
"""EnhancedGCN on 8 Trainium2 NeuronCores (Bass/Tile, SPMD).

Strategy: 1D node partition (6250 nodes/core, padded to 6272). Small weights
replicated. Per propagation step each core computes its shard of the gather
table (step 0: xws0 = dis * (X @ W2) with W2 = conv_w @ lin_w pre-fused
host-side, so no transpose/conv chain; the lin-bias message term is rank-1
and folds into the epilogue via a host-precomputed per-node scalar), then
AllGathers the bf16 table in 4 chunks (partition-major row layout so all
table/lx DMAs move multi-KB contiguous runs), gathers source rows per edge
(dma_gather over 4 SWDGE queues), reduces them into per-target sums with 0/1
selection-matrix matmuls accumulating in PSUM (self-loops enter as an
identity-matmul block), and applies the pointwise epilogue (degree norm,
root/relu term, residual+LN between steps). Edge weights ew = dis[t]*dis[s]
are separable: dis[s] pre-scales the table, dis[t] post-scales the message
sum. The finished output streams out per group in partition-major layout
(host reassembles). Host-side work is limited to graph-structure prep and
weight transposes/fusion.
"""
import sys

sys.path.insert(0, "/opt/trn_rl_repo")

import numpy as np
import ml_dtypes

import concourse.bass as bass
import concourse.bacc as bacc
import concourse.tile as tile
import concourse.mybir as mybir
from concourse.bass_utils import run_bass_kernel_spmd
from concourse.masks import make_identity

BF16 = ml_dtypes.bfloat16
N, IN, H = 50000, 256, 128
NCORES = 8
NPC = N // NCORES  # 6250
NW = (NPC + 127) // 128  # 49
PADN = NW * 128  # 6272
LN_EPS = 1e-5
NGRP = (NW + 3) // 4  # 13 groups of 4 windows

# table chunks (windows per chunk) and the two gather super-streams
CHUNK_W = [12, 12, 12, 13]
CHUNK_W0 = [0, 12, 24, 36, 49]
NCHUNK = 4
NSUP = 2  # chunks 0+1 -> super 0 (windows 0..23), chunks 2+3 -> super 1
SUP_OF_CHUNK = [0, 0, 1, 1]
# table row layout per super: [chunkA: 8 ranks x szA | chunkB: 8 ranks x szB]
CHUNK_SZ = [cw * 128 for cw in CHUNK_W]
TBL_ROWS = [8 * (CHUNK_SZ[0] + CHUNK_SZ[1]), 8 * (CHUNK_SZ[2] + CHUNK_SZ[3])]
TB_OFF = [0, 8 * CHUNK_SZ[0], 0, 8 * CHUNK_SZ[2]]

F32 = mybir.dt.float32
BF = mybir.dt.bfloat16
I16 = mybir.dt.int16
AX = mybir.AluOpType
AF = mybir.ActivationFunctionType


def _bcast_mid(ap, n):
    """[128, F] AP -> [128, n, F] with stride-0 middle dim."""
    a = ap.copy()
    a.ap = [a.ap[0], [0, n]] + a.ap[1:]
    return a


def _r3(ap, f):
    return ap.rearrange("p (w f) -> p w f", f=f)


def _wrap_idx(idx):
    """flat idx [n] (n % 16 == 0) -> [128, n/16] int16 wrapped + replicated."""
    n = len(idx)
    t = idx.reshape(n // 16, 16).T.astype(np.int16)
    return np.tile(t, (8, 1))


def _prep_graph(row, col):
    """Graph-structure-only preprocessing (row/col ints)."""
    deg = np.bincount(row, minlength=N).astype(np.float64) + 1.0
    dis_f = 1.0 / np.sqrt(deg)
    dinv_f = 1.0 / deg
    # q_t = dis_t * (sum_{s in N(t)} dis_s + dis_t): rank-1 lin-bias epilogue
    csum = np.bincount(row, weights=dis_f[col], minlength=N) + dis_f
    q_f = dis_f * csum

    core = row // NPC
    src_core = col // NPC
    src_off = col % NPC
    src_w = src_off >> 7
    src_chunk = np.digitize(src_w, CHUNK_W0[1:4])  # 0..3
    src_sup = (src_chunk >= 2).astype(np.int64)
    base = np.asarray(TB_OFF)[src_chunk]
    csz = np.asarray(CHUNK_SZ)[src_chunk]
    cwn = np.asarray(CHUNK_W)[src_chunk]
    w0 = np.asarray(CHUNK_W0)[src_chunk]
    # table rows within a (chunk, core) block are partition-major
    # (row = p*cw + w_local) so the SBUF->DRAM table write is cw contiguous
    # rows per partition instead of one 256B descriptor per row
    src_p = src_off & 127
    src_wl = src_w - w0
    src_idx = base + src_core * csz + src_p * cwn + src_wl

    per_core = []
    counts = np.zeros((NCORES, NW, NSUP), np.int64)
    for k in range(NCORES):
        m = core == k
        tgt = (row[m] - k * NPC).astype(np.int64)
        sidx = src_idx[m]
        ssup = src_sup[m]
        w = tgt >> 7
        order = np.argsort(w, kind="stable")
        tgt, sidx, ssup, w = tgt[order], sidx[order], ssup[order], w[order]
        ents = []
        bounds = np.searchsorted(w, np.arange(NW + 1))
        for wi in range(NW):
            sl = slice(bounds[wi], bounds[wi + 1])
            s_w, t_w, u_w = sidx[sl], tgt[sl] - (wi << 7), ssup[sl]
            by_sup = []
            for s in range(NSUP):
                mm = u_w == s
                by_sup.append((s_w[mm], t_w[mm]))
                counts[k, wi, s] = int(mm.sum())
            ents.append(by_sup)
        per_core.append(ents)

    B = np.ceil(counts.max(axis=0) / 128).astype(np.int64)  # [NW, NSUP]
    NBLK = B.sum(axis=0).astype(np.int64)  # per super
    gpos = np.zeros((NSUP, NGRP + 1), np.int64)
    for s in range(NSUP):
        pref = np.concatenate([[0], np.cumsum(B[:, s])])
        for g in range(NGRP + 1):
            gpos[s, g] = pref[min(g * 4, NW)]

    rng = np.random.default_rng(12345)
    idx_streams = [np.empty((NCORES, int(NBLK[s]) * 128), np.int64) for s in range(NSUP)]
    for s in range(NSUP):
        idx_streams[s][:] = rng.integers(0, TBL_ROWS[s], idx_streams[s].shape)
    tlocs = [np.full((NCORES, 128, int(NBLK[s])), -1.0, np.float32) for s in range(NSUP)]

    for k in range(NCORES):
        pos = [0] * NSUP
        for wi in range(NW):
            for s in range(NSUP):
                s_w, t_w = per_core[k][wi][s]
                n = len(s_w)
                p = pos[s]
                idx_streams[s][k, p * 128 : p * 128 + n] = s_w
                j = np.arange(n)
                tlocs[s][k, j % 128, p + j // 128] = t_w
                pos[s] += int(B[wi, s])

    # wrap idx per half-call segment (aligned to group boundaries, split in two)
    idx_w = [None] * NSUP
    for s in range(NSUP):
        per_core_w = [[] for _ in range(NCORES)]
        for g in range(NGRP):
            b0, b1 = int(gpos[s, g]), int(gpos[s, g + 1])
            mid = b0 + (b1 - b0 + 1) // 2
            for (h0_, h1_) in ((b0, mid), (mid, b1)):
                if h1_ > h0_:
                    for k in range(NCORES):
                        per_core_w[k].append(
                            _wrap_idx(idx_streams[s][k, h0_ * 128 : h1_ * 128])
                        )
        idx_w[s] = np.stack([np.concatenate(x, axis=1) for x in per_core_w])

    dis_cols = np.zeros((NCORES, 128, NW), np.float32)
    dinv_cols = np.ones((NCORES, 128, NW), np.float32)
    q_cols = np.zeros((NCORES, 128, NW), np.float32)
    dis_full = np.zeros((128, NCORES * NW), np.float32)
    for k in range(NCORES):
        v = np.zeros(PADN, np.float64)
        v[:NPC] = dis_f[k * NPC : (k + 1) * NPC]
        dis_cols[k] = v.reshape(NW, 128).T
        dis_full[:, k * NW : (k + 1) * NW] = dis_cols[k]
        u = np.ones(PADN, np.float64)
        u[:NPC] = dinv_f[k * NPC : (k + 1) * NPC]
        dinv_cols[k] = u.reshape(NW, 128).T
        qv = np.zeros(PADN, np.float64)
        qv[:NPC] = q_f[k * NPC : (k + 1) * NPC]
        q_cols[k] = qv.reshape(NW, 128).T

    return dict(
        B=B,
        NBLK=NBLK,
        gpos=gpos,
        idx_w=idx_w,
        tlocs=[t.astype(BF16) for t in tlocs],
        dis_cols=dis_cols,
        dinv_cols=dinv_cols,
        q_cols=q_cols,
        dis_full=dis_full,
    )


def _build(B, gpos, NBLK):
    nc = bacc.Bacc("TRN2", target_bir_lowering=False, debug=False, num_swdge_queues=4)

    iftl = nc.dram_tensor("iftl", [IN, PADN], BF, kind="ExternalInput")
    lin_wT = nc.dram_tensor("lin_wT", [IN, H], BF, kind="ExternalInput")
    w2T = nc.dram_tensor("w2T", [IN, H], BF, kind="ExternalInput")
    conv_wT = nc.dram_tensor("conv_wT", [H, H], BF, kind="ExternalInput")
    consts = nc.dram_tensor("consts", [128, 6 * H], F32, kind="ExternalInput")
    iota_in = nc.dram_tensor("iota", [128, 128], BF, kind="ExternalInput")
    discols = nc.dram_tensor("discols", [128, NW], F32, kind="ExternalInput")
    dinvcols = nc.dram_tensor("dinvcols", [128, NW], F32, kind="ExternalInput")
    qcols = nc.dram_tensor("qcols", [128, NW], F32, kind="ExternalInput")
    idx_t = [
        nc.dram_tensor(f"idx{s}", [128, int(NBLK[s]) * 8], I16, kind="ExternalInput")
        for s in range(NSUP)
    ]
    tloc_t = [
        nc.dram_tensor(f"tloc{s}", [128, int(NBLK[s])], BF, kind="ExternalInput")
        for s in range(NSUP)
    ]
    # partition-major output: out[p, w*H+f] = h[w*128+p, f]; host reassembles
    out_ext = nc.dram_tensor("out", [128, PADN], F32, kind="ExternalOutput")

    def ws(w):
        return slice(w * 128, (w + 1) * 128)

    # per-(stream, group) half-call boundaries + column offset into wrapped idx
    halves = {}
    for s in range(NSUP):
        off = 0
        for g in range(NGRP):
            b0, b1 = int(gpos[s, g]), int(gpos[s, g + 1])
            mid = b0 + (b1 - b0 + 1) // 2
            hs = []
            for (h0_, h1_) in ((b0, mid), (mid, b1)):
                hs.append((h0_, h1_, off))
                off += (h1_ - h0_) * 8
            halves[(s, g)] = hs
    gmax = max(h1 - h0 for v in halves.values() for (h0, h1, _) in v)
    wmax = int(B.sum(axis=1).max())
    pw = np.concatenate([np.zeros((1, NSUP), np.int64), np.cumsum(B, axis=0)], axis=0)

    with tile.TileContext(nc) as tc:
        with (
            tc.tile_pool(name="const", bufs=1) as cpool,
            tc.tile_pool(name="state", bufs=1) as spool,
            tc.tile_pool(name="iftp", bufs=2) as ipool,
            tc.tile_pool(name="lftp", bufs=4) as lpool,
            tc.tile_pool(name="ht", bufs=4) as hpool,
            tc.tile_pool(name="gath", bufs=14) as gpool,
            tc.tile_pool(name="wp", bufs=5) as wpool,
            tc.tile_pool(name="tmp", bufs=1) as tpool,
            tc.tile_pool(name="psA", bufs=4, space="PSUM") as psA,
            tc.tile_pool(name="psM", bufs=4, space="PSUM") as psM,
            tc.tile_pool(name="dram", bufs=1, space="DRAM") as dpool,
        ):
            identf = cpool.tile([128, 128], F32)
            make_identity(nc, identf[:])
            identb = cpool.tile([128, 128], BF)
            nc.vector.tensor_copy(out=identb[:], in_=identf[:])
            cst = cpool.tile([128, 6 * H], F32)
            nc.sync.dma_start(out=cst[:], in_=consts[:])
            linb, rootr, convbr, g1r, b1r, b2r = (
                cst[:, i * H : (i + 1) * H] for i in range(6)
            )
            iot = cpool.tile([128, 128], BF)
            nc.sync.dma_start(out=iot[:], in_=iota_in[:])
            cw = cpool.tile([128, H], BF)
            nc.sync.dma_start(out=cw[:], in_=conv_wT[:])
            lw0 = cpool.tile([128, H], BF)
            nc.sync.dma_start(out=lw0[:], in_=lin_wT[0:128, :])
            lw1 = cpool.tile([128, H], BF)
            nc.sync.dma_start(out=lw1[:], in_=lin_wT[128:256, :])
            w20 = cpool.tile([128, H], BF)
            nc.sync.dma_start(out=w20[:], in_=w2T[0:128, :])
            w21 = cpool.tile([128, H], BF)
            nc.sync.dma_start(out=w21[:], in_=w2T[128:256, :])
            dic = cpool.tile([128, NW], F32)
            nc.sync.dma_start(out=dic[:], in_=discols[:])
            dvc = cpool.tile([128, NW], F32)
            nc.sync.dma_start(out=dvc[:], in_=dinvcols[:])
            qcl = cpool.tile([128, NW], F32)
            nc.sync.dma_start(out=qcl[:], in_=qcols[:])
            tl_sb = []
            idx_sb = []
            for s in range(NSUP):
                t = cpool.tile([128, int(NBLK[s])], BF, name=f"tl{s}")
                nc.sync.dma_start(out=t[:], in_=tloc_t[s][:])
                tl_sb.append(t)
                t2 = cpool.tile([128, int(NBLK[s]) * 8], I16, name=f"ix{s}")
                nc.sync.dma_start(out=t2[:], in_=idx_t[s][:])
                idx_sb.append(t2)

            h0 = spool.tile([128, PADN], F32, tag="h0")
            hA = spool.tile([128, PADN], F32, tag="hA")
            hB = spool.tile([128, PADN], F32, tag="hB")
            xws0 = spool.tile([128, PADN], BF, tag="xws0")
            xws1 = spool.tile([128, PADN], BF, tag="xws1")

            # ---- step-0: local h0 (lin) + xws0 = dis*(X@W2); publish chunks via AG ----
            tb0 = [
                dpool.tile([TBL_ROWS[0], H], BF, tag="tb0_0", name="tb0A"),
                dpool.tile([TBL_ROWS[1], H], BF, tag="tb0_1", name="tb0B"),
            ]

            def emit_publish0(c):
                w0c, w1c = CHUNK_W0[c], CHUNK_W0[c + 1]
                for w in range(w0c, w1c):
                    i0 = lpool.tile([128, 128], BF, tag="lfta")
                    nc.sync.dma_start(out=i0[:], in_=iftl[0:128, ws(w)])
                    i1 = lpool.tile([128, 128], BF, tag="lftb")
                    nc.sync.dma_start(out=i1[:], in_=iftl[128:256, ws(w)])
                    xp = psA.tile([128, 128], F32, tag="ps128")
                    nc.tensor.matmul(xp[:], lhsT=i0[:], rhs=w20[:], start=True, stop=False)
                    nc.tensor.matmul(xp[:], lhsT=i1[:], rhs=w21[:], start=False, stop=True)
                    nc.scalar.activation(
                        out=xws0[:, ws(w)], in_=xp[:], func=AF.Copy, scale=dic[:, w : w + 1]
                    )
                    hp = psA.tile([128, 128], F32, tag="ps128")
                    nc.tensor.matmul(hp[:], lhsT=i0[:], rhs=lw0[:], start=True, stop=False)
                    nc.tensor.matmul(hp[:], lhsT=i1[:], rhs=lw1[:], start=False, stop=True)
                    nc.vector.tensor_tensor(out=h0[:, ws(w)], in0=hp[:], in1=linb, op=AX.add)
                csz = CHUNK_SZ[c]
                lx = dpool.tile([csz, H], BF, tag=f"lx0_{c}", name=f"lx0_{c}")
                nc.sync.dma_start(
                    out=lx[:].rearrange("(p w) f -> p w f", w=w1c - w0c),
                    in_=_r3(xws0[:, w0c * 128 : w1c * 128], 128),
                )
                sup = SUP_OF_CHUNK[c]
                dst = tb0[sup][:][TB_OFF[c] : TB_OFF[c] + 8 * csz, :]
                nc.gpsimd.collective_compute(
                    "AllGather",
                    AX.bypass,
                    replica_groups=[list(range(NCORES))],
                    ins=[lx.opt()],
                    outs=[dst],
                )

            ctxs = {
                0: dict(tb=tb0, call_tiles={}, w_tiles={}, blkpos=[0] * NSUP),
            }

            def emit_publish(c):
                """Step-1: LN + xws1 for chunk c + publish + AllGather (Shared out)."""
                if 1 not in ctxs:
                    tbA = dpool.tile([TBL_ROWS[0], H], BF, tag="tb1_0", name="tb1A")
                    tbB = dpool.tile([TBL_ROWS[1], H], BF, tag="tb1_1", name="tb1B")
                    ctxs[1] = dict(tb=[tbA, tbB], call_tiles={}, w_tiles={}, blkpos=[0] * NSUP)
                ctx = ctxs[1]
                w0c, w1c = CHUNK_W0[c], CHUNK_W0[c + 1]
                # residual + layernorm + relu -> hB for this chunk's windows
                for g in range(w0c, w1c, 4):
                    gw = min(4, w1c - g)
                    sl = slice(g * 128, (g + gw) * 128)
                    X_t = tpool.tile([128, 4 * 128], F32, tag="ln_X")
                    X = X_t[:, : gw * 128]
                    Y_t = tpool.tile([128, 4 * 128], F32, tag="ln_Y")
                    Y = Y_t[:, : gw * 128]
                    nc.vector.tensor_tensor(out=X, in0=hA[:, sl], in1=h0[:, sl], op=AX.add)
                    mu_t = tpool.tile([128, 4], F32, tag="ln_mu")
                    mu = mu_t[:, :gw]
                    nc.vector.tensor_reduce(out=mu, in_=_r3(X, 128), axis=mybir.AxisListType.X, op=AX.add)
                    nc.vector.tensor_scalar_mul(out=mu, in0=mu, scalar1=1.0 / 128.0)
                    nc.vector.tensor_tensor(out=Y, in0=X, in1=X, op=AX.mult)
                    var_t = tpool.tile([128, 4], F32, tag="ln_var")
                    var = var_t[:, :gw]
                    nc.vector.tensor_reduce(out=var, in_=_r3(Y, 128), axis=mybir.AxisListType.X, op=AX.add)
                    mm_t = tpool.tile([128, 4], F32, tag="ln_mm")
                    mm = mm_t[:, :gw]
                    nc.vector.tensor_tensor(out=mm, in0=mu, in1=mu, op=AX.mult)
                    nc.vector.tensor_scalar(
                        out=var, in0=var, scalar1=1.0 / 128.0, scalar2=LN_EPS, op0=AX.mult, op1=AX.add
                    )
                    nc.vector.tensor_tensor(out=var, in0=var, in1=mm, op=AX.subtract)
                    sd_t = tpool.tile([128, 4], F32, tag="ln_sd")
                    sd = sd_t[:, :gw]
                    nc.scalar.activation(out=sd, in_=var, func=AF.Sqrt)
                    rstd_t = tpool.tile([128, 4], F32, tag="ln_rs")
                    rstd = rstd_t[:, :gw]
                    nc.vector.reciprocal(out=rstd, in_=sd)
                    mb_t = tpool.tile([128, 4], F32, tag="ln_mb")
                    mb = mb_t[:, :gw]
                    nc.vector.tensor_tensor(out=mb, in0=mu, in1=rstd, op=AX.mult)
                    nc.vector.tensor_scalar_mul(out=mb, in0=mb, scalar1=-1.0)
                    for wq in range(gw):
                        nc.scalar.activation(
                            out=X_t[:, wq * 128 : (wq + 1) * 128],
                            in_=X_t[:, wq * 128 : (wq + 1) * 128],
                            func=AF.Identity,
                            scale=rstd_t[:, wq : wq + 1],
                            bias=mb_t[:, wq : wq + 1],
                        )
                    nc.vector.tensor_tensor(out=_r3(Y, 128), in0=_r3(X, 128), in1=_bcast_mid(g1r, gw), op=AX.mult)
                    nc.vector.tensor_tensor(out=_r3(X, 128), in0=_r3(Y, 128), in1=_bcast_mid(b1r, gw), op=AX.add)
                    nc.scalar.activation(out=hB[:, sl], in_=X, func=AF.Relu)
                # xws1 = dis * (hB @ conv_w.T) for this chunk's windows
                for w in range(w0c, w1c):
                    tp = psA.tile([128, 128], F32, tag="ps128")
                    nc.tensor.transpose(tp[:], hB[:, ws(w)], identf[:])
                    ht = hpool.tile([128, 128], BF, tag="ht")
                    nc.scalar.copy(out=ht[:], in_=tp[:])
                    xp = psA.tile([128, 128], F32, tag="ps128")
                    nc.tensor.matmul(xp[:], lhsT=ht[:], rhs=cw[:], start=True, stop=True)
                    nc.scalar.activation(
                        out=xws1[:, ws(w)], in_=xp[:], func=AF.Copy, scale=dic[:, w : w + 1]
                    )
                csz = CHUNK_SZ[c]
                lx = dpool.tile([csz, H], BF, tag=f"lx{c}", name=f"lx{c}")
                nc.sync.dma_start(
                    out=lx[:].rearrange("(p w) f -> p w f", w=w1c - w0c),
                    in_=_r3(xws1[:, w0c * 128 : w1c * 128], 128),
                )
                sup = SUP_OF_CHUNK[c]
                dst = ctx["tb"][sup][:][TB_OFF[c] : TB_OFF[c] + 8 * csz, :]
                nc.gpsimd.collective_compute(
                    "AllGather",
                    AX.bypass,
                    replica_groups=[list(range(NCORES))],
                    ins=[lx.opt()],
                    outs=[dst],
                )

            def call_tile(s_step, s, g, h):
                ctx = ctxs[s_step]
                key = (s, g, h)
                if key not in ctx["call_tiles"]:
                    h0_, h1_, off = halves[(s, g)][h]
                    nb = h1_ - h0_
                    if nb == 0:
                        ctx["call_tiles"][key] = None
                    else:
                        gt = gpool.tile([128, gmax * H], BF, tag="gath")
                        nc.gpsimd.dma_gather(
                            gt[:, : nb * H].rearrange("p (b e) -> p b e", e=H),
                            ctx["tb"][s][:],
                            idx_sb[s][:, off : off + nb * 8],
                            nb * 128,
                            nb * 128,
                            H,
                            single_packet=False,
                            queue_num=s * 2 + h,
                        )
                        ctx["call_tiles"][key] = gt
                return ctx["call_tiles"][key]

            def prefetch(s_step, grp):
                for gg in range(grp, min(grp + 3, NGRP)):
                    for s in range(NSUP):
                        call_tile(s_step, s, gg, 0)
                        call_tile(s_step, s, gg, 1)

            def w_tile(s_step, w):
                ctx = ctxs[s_step]
                if w not in ctx["w_tiles"]:
                    wt = wpool.tile([128, wmax * 128], BF, tag="W")
                    offs = []
                    o = 0
                    for s in range(NSUP):
                        nb = int(B[w, s])
                        offs.append(o)
                        if nb > 0:
                            nc.vector.tensor_tensor(
                                out=_r3(wt[:, o * 128 : (o + nb) * 128], 128),
                                in0=tl_sb[s][:, int(pw[w, s]) : int(pw[w + 1, s])].to_broadcast([128, nb, 128]),
                                in1=_bcast_mid(iot[:], nb),
                                op=AX.is_equal,
                            )
                        o += nb
                    ctx["w_tiles"][w] = (wt, offs)
                return ctx["w_tiles"][w]

            def emit_groups(s_step, glo, ghi):
                ctx = ctxs[s_step]
                state = hB if s_step == 1 else h0
                xws_s = xws1 if s_step == 1 else xws0
                hdst = hA
                for grp in range(glo, ghi):
                    bg = grp * 4
                    prefetch(s_step, grp)
                    gw = min(4, NW - bg)
                    pm = psM.tile([128, 4 * 128], F32, tag="msg")
                    for wq in range(gw):
                        w = bg + wq
                        dst = pm[:, wq * 128 : (wq + 1) * 128]
                        nc.tensor.matmul(dst, lhsT=identb[:], rhs=xws_s[:, ws(w)], start=True, stop=False)
                        nblk = int(B[w].sum())
                        bi = 0
                        for s in range(NSUP):
                            for _ in range(int(B[w, s])):
                                gidx = ctx["blkpos"][s]
                                hh = halves[(s, grp)]
                                h = 0 if gidx < hh[0][1] else 1
                                h0_, h1_, _off = hh[h]
                                ct = call_tile(s_step, s, grp, h)
                                loc = gidx - h0_
                                wt_, woffs = w_tile(s_step, w)
                                wloc = woffs[s] + (gidx - int(pw[w, s]))
                                nc.tensor.matmul(
                                    dst,
                                    lhsT=wt_[:, wloc * 128 : (wloc + 1) * 128],
                                    rhs=ct[:].rearrange("p (b e) -> p b e", e=H)[:, loc, :],
                                    start=False,
                                    stop=(bi == nblk - 1),
                                )
                                ctx["blkpos"][s] += 1
                                bi += 1
                    sl = slice(bg * 128, (bg + gw) * 128)
                    E1_t = tpool.tile([128, 4 * 128], F32, tag="ep_E1")
                    E1 = E1_t[:, : gw * 128]
                    E2_t = tpool.tile([128, 4 * 128], F32, tag="ep_E2")
                    E2 = E2_t[:, : gw * 128]
                    E3_t = tpool.tile([128, 4 * 128], F32, tag="ep_E3")
                    E3 = E3_t[:, : gw * 128]
                    nc.vector.tensor_tensor(
                        out=_r3(E1, 128), in0=_r3(state[:, sl], 128), in1=_bcast_mid(rootr, gw), op=AX.add
                    )
                    for wq in range(gw):
                        w = bg + wq
                        nc.scalar.activation(
                            out=E2_t[:, wq * 128 : (wq + 1) * 128],
                            in_=E1_t[:, wq * 128 : (wq + 1) * 128],
                            func=AF.Relu,
                            scale=dvc[:, w : w + 1],
                        )
                        nc.scalar.activation(
                            out=E3_t[:, wq * 128 : (wq + 1) * 128],
                            in_=pm[:, wq * 128 : (wq + 1) * 128],
                            func=AF.Copy,
                            scale=dic[:, w : w + 1],
                        )
                    nc.vector.tensor_tensor(out=E2, in0=E3, in1=E2, op=AX.add)
                    if s_step == 0:
                        # rank-1 lin-bias message term: + q_t * b2
                        for wq in range(gw):
                            w = bg + wq
                            nc.scalar.activation(
                                out=E3_t[:, wq * 128 : (wq + 1) * 128],
                                in_=b2r,
                                func=AF.Copy,
                                scale=qcl[:, w : w + 1],
                            )
                        nc.vector.tensor_tensor(out=E2, in0=E3, in1=E2, op=AX.add)
                    nc.vector.tensor_tensor(
                        out=_r3(hdst[:, sl], 128), in0=_r3(E2, 128), in1=_bcast_mid(convbr, gw), op=AX.add
                    )
                    if s_step == 1:
                        # stream the finished group straight out (contiguous
                        # per-partition rows; H == 128 makes layouts identical)
                        nc.sync.dma_start(out=out_ext[:, sl], in_=hdst[:, sl])

            # software-pipelined emission: step-1 publishes overlap step-0 consumption
            for c in range(NCHUNK):
                emit_publish0(c)
            emit_groups(0, 0, 3)
            emit_publish(0)
            emit_groups(0, 3, 6)
            emit_publish(1)
            emit_groups(0, 6, 9)
            emit_publish(2)
            emit_groups(0, 9, NGRP)
            emit_publish(3)
            emit_groups(1, 0, NGRP)
    nc.compile()
    return nc


def _rep(v):
    return np.tile(np.asarray(v, np.float32).reshape(1, H), (128, 1))


def kernel_with_results(**inputs):
    in_feat = np.asarray(inputs["in_feat"], np.float32)
    row = np.asarray(inputs["row"]).astype(np.int64)
    col = np.asarray(inputs["col"]).astype(np.int64)
    lin_w = np.asarray(inputs["lin_w"], np.float32)
    lin_b = np.asarray(inputs["lin_b"], np.float32)
    conv_w = np.asarray(inputs["conv_w"], np.float32)
    conv_b = np.asarray(inputs["conv_b"], np.float32)
    root_emb = np.asarray(inputs["root_emb"], np.float32)
    ln_gamma = np.asarray(inputs["ln_gamma"], np.float32)
    ln_beta = np.asarray(inputs["ln_beta"], np.float32)

    g = _prep_graph(row, col)
    nc = _build(g["B"], g["gpos"], g["NBLK"])

    ift_t = in_feat.T  # [IN, N]
    # fused step-0 weights: xw0 = X @ W2 + b2, W2 = (conv_w @ lin_w).T
    w2 = (conv_w.astype(np.float64) @ lin_w.astype(np.float64)).astype(np.float32)
    b2 = (conv_w.astype(np.float64) @ lin_b.astype(np.float64)).astype(np.float32)
    consts = np.concatenate(
        [_rep(lin_b), _rep(root_emb[0]), _rep(conv_b), _rep(ln_gamma[1]), _rep(ln_beta[1]), _rep(b2)],
        axis=1,
    )
    iota = np.tile(np.arange(128, dtype=np.float32), (128, 1)).astype(BF16)
    lin_wT = np.ascontiguousarray(lin_w.T).astype(BF16)
    w2T = np.ascontiguousarray(w2.T).astype(BF16)
    conv_wT = np.ascontiguousarray(conv_w.T).astype(BF16)

    in_maps = []
    for k in range(NCORES):
        ift_k = np.zeros((IN, PADN), BF16)
        ift_k[:, :NPC] = ift_t[:, k * NPC : (k + 1) * NPC].astype(BF16)
        m = {
            "iftl": ift_k,
            "lin_wT": lin_wT,
            "w2T": w2T,
            "conv_wT": conv_wT,
            "consts": consts,
            "iota": iota,
            "discols": g["dis_cols"][k],
            "dinvcols": g["dinv_cols"][k],
            "qcols": g["q_cols"][k],
        }
        for s in range(NSUP):
            m[f"idx{s}"] = g["idx_w"][s][k]
            m[f"tloc{s}"] = np.ascontiguousarray(g["tlocs"][s][k])
        in_maps.append(m)

    res = run_bass_kernel_spmd(nc, in_maps, list(range(NCORES)))
    shards = []
    for k in range(NCORES):
        o = np.asarray(res.results[k]["out"])  # [128, NW*H], [p, w*H+f]
        o = o.reshape(128, NW, H).transpose(1, 0, 2).reshape(PADN, H)
        shards.append(o[:NPC])
    out = np.concatenate(shards, axis=0)
    return out.astype(np.float32), res


def kernel(**inputs):
    out, _ = kernel_with_results(**inputs)
    return out


# revision 28
# speedup vs baseline: 1.2125x; 1.0056x over previous
"""EnhancedGCN on 8 Trainium2 NeuronCores (Bass/Tile, SPMD).

Strategy: 1D node partition (6250 nodes/core, padded to 6272). Small weights
replicated. Per propagation step each core computes its shard of the gather
table (step 0: xws0 = dis * (X @ W2) with W2 = conv_w @ lin_w pre-fused
host-side, so no transpose/conv chain; the lin-bias message term is rank-1
and folds into the epilogue via a host-precomputed per-node scalar), then
AllGathers the bf16 table in 4 chunks (partition-major row layout so all
table/lx DMAs move multi-KB contiguous runs), gathers source rows per edge
(dma_gather over 4 SWDGE queues), reduces them into per-target sums with 0/1
selection-matrix matmuls accumulating in PSUM (self-loops enter as an
identity-matmul block), and applies the pointwise epilogue (degree norm,
root/relu term, residual+LN between steps). Edge weights ew = dis[t]*dis[s]
are separable: dis[s] pre-scales the table, dis[t] post-scales the message
sum. The finished output streams out per group in partition-major layout
(host reassembles). Host-side work is limited to graph-structure prep and
weight transposes/fusion.
"""
import sys

sys.path.insert(0, "/opt/trn_rl_repo")

import numpy as np
import ml_dtypes

import concourse.bass as bass
import concourse.bacc as bacc
import concourse.tile as tile
import concourse.mybir as mybir
from concourse.bass_utils import run_bass_kernel_spmd
from concourse.masks import make_identity

BF16 = ml_dtypes.bfloat16
N, IN, H = 50000, 256, 128
NCORES = 8
NPC = N // NCORES  # 6250
NW = (NPC + 127) // 128  # 49
PADN = NW * 128  # 6272
LN_EPS = 1e-5
NGRP = (NW + 3) // 4  # 13 groups of 4 windows

# table chunks (windows per chunk) and the two gather super-streams
CHUNK_W = [12, 12, 12, 13]
CHUNK_W0 = [0, 12, 24, 36, 49]
NCHUNK = 4
NSUP = 2  # chunks 0+1 -> super 0 (windows 0..23), chunks 2+3 -> super 1
SUP_OF_CHUNK = [0, 0, 1, 1]
# table row layout per super: [chunkA: 8 ranks x szA | chunkB: 8 ranks x szB]
CHUNK_SZ = [cw * 128 for cw in CHUNK_W]
TBL_ROWS = [8 * (CHUNK_SZ[0] + CHUNK_SZ[1]), 8 * (CHUNK_SZ[2] + CHUNK_SZ[3])]
TB_OFF = [0, 8 * CHUNK_SZ[0], 0, 8 * CHUNK_SZ[2]]

F32 = mybir.dt.float32
BF = mybir.dt.bfloat16
I16 = mybir.dt.int16
AX = mybir.AluOpType
AF = mybir.ActivationFunctionType


def _bcast_mid(ap, n):
    """[128, F] AP -> [128, n, F] with stride-0 middle dim."""
    a = ap.copy()
    a.ap = [a.ap[0], [0, n]] + a.ap[1:]
    return a


def _r3(ap, f):
    return ap.rearrange("p (w f) -> p w f", f=f)


def _wrap_idx(idx):
    """flat idx [n] (n % 16 == 0) -> [128, n/16] int16 wrapped + replicated."""
    n = len(idx)
    t = idx.reshape(n // 16, 16).T.astype(np.int16)
    return np.tile(t, (8, 1))


def _prep_graph(row, col):
    """Graph-structure-only preprocessing (row/col ints)."""
    deg = np.bincount(row, minlength=N).astype(np.float64) + 1.0
    dis_f = 1.0 / np.sqrt(deg)
    dinv_f = 1.0 / deg
    # q_t = dis_t * (sum_{s in N(t)} dis_s + dis_t): rank-1 lin-bias epilogue
    csum = np.bincount(row, weights=dis_f[col], minlength=N) + dis_f
    q_f = dis_f * csum

    core = row // NPC
    src_core = col // NPC
    src_off = col % NPC
    src_w = src_off >> 7
    src_chunk = np.digitize(src_w, CHUNK_W0[1:4])  # 0..3
    src_sup = (src_chunk >= 2).astype(np.int64)
    base = np.asarray(TB_OFF)[src_chunk]
    csz = np.asarray(CHUNK_SZ)[src_chunk]
    cwn = np.asarray(CHUNK_W)[src_chunk]
    w0 = np.asarray(CHUNK_W0)[src_chunk]
    # table rows within a (chunk, core) block are partition-major
    # (row = p*cw + w_local) so the SBUF->DRAM table write is cw contiguous
    # rows per partition instead of one 256B descriptor per row
    src_p = src_off & 127
    src_wl = src_w - w0
    src_idx = base + src_core * csz + src_p * cwn + src_wl

    per_core = []
    counts = np.zeros((NCORES, NW, NSUP), np.int64)
    for k in range(NCORES):
        m = core == k
        tgt = (row[m] - k * NPC).astype(np.int64)
        sidx = src_idx[m]
        ssup = src_sup[m]
        w = tgt >> 7
        order = np.argsort(w, kind="stable")
        tgt, sidx, ssup, w = tgt[order], sidx[order], ssup[order], w[order]
        ents = []
        bounds = np.searchsorted(w, np.arange(NW + 1))
        for wi in range(NW):
            sl = slice(bounds[wi], bounds[wi + 1])
            s_w, t_w, u_w = sidx[sl], tgt[sl] - (wi << 7), ssup[sl]
            by_sup = []
            for s in range(NSUP):
                mm = u_w == s
                by_sup.append((s_w[mm], t_w[mm]))
                counts[k, wi, s] = int(mm.sum())
            ents.append(by_sup)
        per_core.append(ents)

    B = np.ceil(counts.max(axis=0) / 128).astype(np.int64)  # [NW, NSUP]
    NBLK = B.sum(axis=0).astype(np.int64)  # per super
    gpos = np.zeros((NSUP, NGRP + 1), np.int64)
    for s in range(NSUP):
        pref = np.concatenate([[0], np.cumsum(B[:, s])])
        for g in range(NGRP + 1):
            gpos[s, g] = pref[min(g * 4, NW)]

    rng = np.random.default_rng(12345)
    idx_streams = [np.empty((NCORES, int(NBLK[s]) * 128), np.int64) for s in range(NSUP)]
    for s in range(NSUP):
        idx_streams[s][:] = rng.integers(0, TBL_ROWS[s], idx_streams[s].shape)
    tlocs = [np.full((NCORES, 128, int(NBLK[s])), -1.0, np.float32) for s in range(NSUP)]

    for k in range(NCORES):
        pos = [0] * NSUP
        for wi in range(NW):
            for s in range(NSUP):
                s_w, t_w = per_core[k][wi][s]
                n = len(s_w)
                p = pos[s]
                idx_streams[s][k, p * 128 : p * 128 + n] = s_w
                j = np.arange(n)
                tlocs[s][k, j % 128, p + j // 128] = t_w
                pos[s] += int(B[wi, s])

    # wrap idx per half-call segment (aligned to group boundaries, split in two)
    idx_w = [None] * NSUP
    for s in range(NSUP):
        per_core_w = [[] for _ in range(NCORES)]
        for g in range(NGRP):
            b0, b1 = int(gpos[s, g]), int(gpos[s, g + 1])
            mid = b0 + (b1 - b0 + 1) // 2
            for (h0_, h1_) in ((b0, mid), (mid, b1)):
                if h1_ > h0_:
                    for k in range(NCORES):
                        per_core_w[k].append(
                            _wrap_idx(idx_streams[s][k, h0_ * 128 : h1_ * 128])
                        )
        idx_w[s] = np.stack([np.concatenate(x, axis=1) for x in per_core_w])

    dis_cols = np.zeros((NCORES, 128, NW), np.float32)
    dinv_cols = np.ones((NCORES, 128, NW), np.float32)
    q_cols = np.zeros((NCORES, 128, NW), np.float32)
    dis_full = np.zeros((128, NCORES * NW), np.float32)
    for k in range(NCORES):
        v = np.zeros(PADN, np.float64)
        v[:NPC] = dis_f[k * NPC : (k + 1) * NPC]
        dis_cols[k] = v.reshape(NW, 128).T
        dis_full[:, k * NW : (k + 1) * NW] = dis_cols[k]
        u = np.ones(PADN, np.float64)
        u[:NPC] = dinv_f[k * NPC : (k + 1) * NPC]
        dinv_cols[k] = u.reshape(NW, 128).T
        qv = np.zeros(PADN, np.float64)
        qv[:NPC] = q_f[k * NPC : (k + 1) * NPC]
        q_cols[k] = qv.reshape(NW, 128).T

    # host-precomputed 0/1 selection matrices (identical for both steps):
    # per window w the blocks are [sup0 j=0..B[w,0) | sup1 j=0..B[w,1)], each
    # a [128, 128] one-hot lhsT (partition = edge slot, free = target-in-window)
    TOTW = int(B.sum())
    woff = np.concatenate([[0], np.cumsum(B.sum(axis=1))]).astype(np.int64)
    pwh = np.concatenate([np.zeros((1, NSUP), np.int64), np.cumsum(B, axis=0)], axis=0)
    ar = np.arange(128, dtype=np.float32)
    wtab = np.zeros((NCORES, 128, TOTW * 128), BF16)
    for k in range(NCORES):
        for w in range(NW):
            o = int(woff[w])
            for s in range(NSUP):
                for j in range(int(B[w, s])):
                    tl = tlocs[s][k][:, int(pwh[w, s]) + j]
                    blk = o + (j if s == 0 else int(B[w, 0]) + j)
                    wtab[k, :, blk * 128 : (blk + 1) * 128] = (tl[:, None] == ar)

    return dict(
        B=B,
        NBLK=NBLK,
        gpos=gpos,
        idx_w=idx_w,
        wtab=wtab,
        woff=woff,
        dis_cols=dis_cols,
        dinv_cols=dinv_cols,
        q_cols=q_cols,
        dis_full=dis_full,
    )


def _build(B, gpos, NBLK, woff):
    nc = bacc.Bacc("TRN2", target_bir_lowering=False, debug=False, num_swdge_queues=4)

    iftl = nc.dram_tensor("iftl", [IN, PADN], BF, kind="ExternalInput")
    lin_wT = nc.dram_tensor("lin_wT", [IN, H], BF, kind="ExternalInput")
    w2T = nc.dram_tensor("w2T", [IN, H], BF, kind="ExternalInput")
    conv_wT = nc.dram_tensor("conv_wT", [H, H], BF, kind="ExternalInput")
    consts = nc.dram_tensor("consts", [128, 6 * H], F32, kind="ExternalInput")
    discols = nc.dram_tensor("discols", [128, NW], F32, kind="ExternalInput")
    dinvcols = nc.dram_tensor("dinvcols", [128, NW], F32, kind="ExternalInput")
    qcols = nc.dram_tensor("qcols", [128, NW], F32, kind="ExternalInput")
    idx_t = [
        nc.dram_tensor(f"idx{s}", [128, int(NBLK[s]) * 8], I16, kind="ExternalInput")
        for s in range(NSUP)
    ]
    TOTW = int(B.sum())
    wtab_t = nc.dram_tensor("wtab", [128, TOTW * 128], BF, kind="ExternalInput")
    # partition-major output: out[p, w*H+f] = h[w*128+p, f]; host reassembles
    out_ext = nc.dram_tensor("out", [128, PADN], F32, kind="ExternalOutput")

    def ws(w):
        return slice(w * 128, (w + 1) * 128)

    # per-(stream, group) half-call boundaries + column offset into wrapped idx
    halves = {}
    for s in range(NSUP):
        off = 0
        for g in range(NGRP):
            b0, b1 = int(gpos[s, g]), int(gpos[s, g + 1])
            mid = b0 + (b1 - b0 + 1) // 2
            hs = []
            for (h0_, h1_) in ((b0, mid), (mid, b1)):
                hs.append((h0_, h1_, off))
                off += (h1_ - h0_) * 8
            halves[(s, g)] = hs
    gmax = max(h1 - h0 for v in halves.values() for (h0, h1, _) in v)
    wgmax = int(max(woff[min(g * 4 + 4, NW)] - woff[g * 4] for g in range(NGRP)))
    pw = np.concatenate([np.zeros((1, NSUP), np.int64), np.cumsum(B, axis=0)], axis=0)

    with tile.TileContext(nc) as tc:
        with (
            tc.tile_pool(name="const", bufs=1) as cpool,
            tc.tile_pool(name="state", bufs=1) as spool,
            tc.tile_pool(name="iftp", bufs=2) as ipool,
            tc.tile_pool(name="lftp", bufs=3) as lpool,
            tc.tile_pool(name="ht", bufs=4) as hpool,
            tc.tile_pool(name="gath", bufs=12) as gpool,
            tc.tile_pool(name="wp", bufs=2) as wpool,
            tc.tile_pool(name="tmp", bufs=1) as tpool,
            tc.tile_pool(name="psA", bufs=4, space="PSUM") as psA,
            tc.tile_pool(name="psM", bufs=4, space="PSUM") as psM,
            tc.tile_pool(name="dram", bufs=1, space="DRAM") as dpool,
        ):
            identf = cpool.tile([128, 128], F32)
            make_identity(nc, identf[:])
            identb = cpool.tile([128, 128], BF)
            nc.vector.tensor_copy(out=identb[:], in_=identf[:])
            cst = cpool.tile([128, 6 * H], F32)
            nc.sync.dma_start(out=cst[:], in_=consts[:])
            linb, rootr, convbr, g1r, b1r, b2r = (
                cst[:, i * H : (i + 1) * H] for i in range(6)
            )
            cw = cpool.tile([128, H], BF)
            nc.sync.dma_start(out=cw[:], in_=conv_wT[:])
            lw0 = cpool.tile([128, H], BF)
            nc.sync.dma_start(out=lw0[:], in_=lin_wT[0:128, :])
            lw1 = cpool.tile([128, H], BF)
            nc.sync.dma_start(out=lw1[:], in_=lin_wT[128:256, :])
            w20 = cpool.tile([128, H], BF)
            nc.sync.dma_start(out=w20[:], in_=w2T[0:128, :])
            w21 = cpool.tile([128, H], BF)
            nc.sync.dma_start(out=w21[:], in_=w2T[128:256, :])
            dic = cpool.tile([128, NW], F32)
            nc.sync.dma_start(out=dic[:], in_=discols[:])
            dvc = cpool.tile([128, NW], F32)
            nc.sync.dma_start(out=dvc[:], in_=dinvcols[:])
            qcl = cpool.tile([128, NW], F32)
            nc.sync.dma_start(out=qcl[:], in_=qcols[:])
            idx_sb = []
            for s in range(NSUP):
                t2 = cpool.tile([128, int(NBLK[s]) * 8], I16, name=f"ix{s}")
                nc.sync.dma_start(out=t2[:], in_=idx_t[s][:])
                idx_sb.append(t2)

            h0 = spool.tile([128, PADN], F32, tag="h0")
            hA = spool.tile([128, PADN], F32, tag="hA")
            hB = spool.tile([128, PADN], F32, tag="hB")
            xws0 = spool.tile([128, PADN], BF, tag="xws0")
            xws1 = spool.tile([128, PADN], BF, tag="xws1")

            # ---- step-0: local h0 (lin) + xws0 = dis*(X@W2); publish chunks via AG ----
            tb0 = [
                dpool.tile([TBL_ROWS[0], H], BF, tag="tb0_0", name="tb0A"),
                dpool.tile([TBL_ROWS[1], H], BF, tag="tb0_1", name="tb0B"),
            ]

            def emit_publish0(c):
                w0c, w1c = CHUNK_W0[c], CHUNK_W0[c + 1]
                for w in range(w0c, w1c):
                    i0 = lpool.tile([128, 128], BF, tag="lfta")
                    nc.sync.dma_start(out=i0[:], in_=iftl[0:128, ws(w)])
                    i1 = lpool.tile([128, 128], BF, tag="lftb")
                    nc.sync.dma_start(out=i1[:], in_=iftl[128:256, ws(w)])
                    xp = psA.tile([128, 128], F32, tag="ps128")
                    nc.tensor.matmul(xp[:], lhsT=i0[:], rhs=w20[:], start=True, stop=False)
                    nc.tensor.matmul(xp[:], lhsT=i1[:], rhs=w21[:], start=False, stop=True)
                    nc.scalar.activation(
                        out=xws0[:, ws(w)], in_=xp[:], func=AF.Copy, scale=dic[:, w : w + 1]
                    )
                    hp = psA.tile([128, 128], F32, tag="ps128")
                    nc.tensor.matmul(hp[:], lhsT=i0[:], rhs=lw0[:], start=True, stop=False)
                    nc.tensor.matmul(hp[:], lhsT=i1[:], rhs=lw1[:], start=False, stop=True)
                    nc.vector.tensor_tensor(out=h0[:, ws(w)], in0=hp[:], in1=linb, op=AX.add)
                csz = CHUNK_SZ[c]
                lx = dpool.tile([csz, H], BF, tag=f"lx0_{c}", name=f"lx0_{c}")
                nc.sync.dma_start(
                    out=lx[:].rearrange("(p w) f -> p w f", w=w1c - w0c),
                    in_=_r3(xws0[:, w0c * 128 : w1c * 128], 128),
                )
                sup = SUP_OF_CHUNK[c]
                dst = tb0[sup][:][TB_OFF[c] : TB_OFF[c] + 8 * csz, :]
                nc.gpsimd.collective_compute(
                    "AllGather",
                    AX.bypass,
                    replica_groups=[list(range(NCORES))],
                    ins=[lx.opt()],
                    outs=[dst],
                )

            ctxs = {
                0: dict(tb=tb0, call_tiles={}, w_tiles={}, blkpos=[0] * NSUP),
            }

            def emit_publish(c):
                """Step-1: LN + xws1 for chunk c + publish + AllGather (Shared out)."""
                if 1 not in ctxs:
                    tbA = dpool.tile([TBL_ROWS[0], H], BF, tag="tb1_0", name="tb1A")
                    tbB = dpool.tile([TBL_ROWS[1], H], BF, tag="tb1_1", name="tb1B")
                    ctxs[1] = dict(tb=[tbA, tbB], call_tiles={}, w_tiles={}, blkpos=[0] * NSUP)
                ctx = ctxs[1]
                w0c, w1c = CHUNK_W0[c], CHUNK_W0[c + 1]
                # residual + layernorm + relu -> hB for this chunk's windows
                for g in range(w0c, w1c, 4):
                    gw = min(4, w1c - g)
                    sl = slice(g * 128, (g + gw) * 128)
                    X_t = tpool.tile([128, 4 * 128], F32, tag="ln_X")
                    X = X_t[:, : gw * 128]
                    Y_t = tpool.tile([128, 4 * 128], F32, tag="ln_Y")
                    Y = Y_t[:, : gw * 128]
                    nc.vector.tensor_tensor(out=X, in0=hA[:, sl], in1=h0[:, sl], op=AX.add)
                    mu_t = tpool.tile([128, 4], F32, tag="ln_mu")
                    mu = mu_t[:, :gw]
                    nc.vector.tensor_reduce(out=mu, in_=_r3(X, 128), axis=mybir.AxisListType.X, op=AX.add)
                    nc.vector.tensor_scalar_mul(out=mu, in0=mu, scalar1=1.0 / 128.0)
                    nc.vector.tensor_tensor(out=Y, in0=X, in1=X, op=AX.mult)
                    var_t = tpool.tile([128, 4], F32, tag="ln_var")
                    var = var_t[:, :gw]
                    nc.vector.tensor_reduce(out=var, in_=_r3(Y, 128), axis=mybir.AxisListType.X, op=AX.add)
                    mm_t = tpool.tile([128, 4], F32, tag="ln_mm")
                    mm = mm_t[:, :gw]
                    nc.vector.tensor_tensor(out=mm, in0=mu, in1=mu, op=AX.mult)
                    nc.vector.tensor_scalar(
                        out=var, in0=var, scalar1=1.0 / 128.0, scalar2=LN_EPS, op0=AX.mult, op1=AX.add
                    )
                    nc.vector.tensor_tensor(out=var, in0=var, in1=mm, op=AX.subtract)
                    sd_t = tpool.tile([128, 4], F32, tag="ln_sd")
                    sd = sd_t[:, :gw]
                    nc.scalar.activation(out=sd, in_=var, func=AF.Sqrt)
                    rstd_t = tpool.tile([128, 4], F32, tag="ln_rs")
                    rstd = rstd_t[:, :gw]
                    nc.vector.reciprocal(out=rstd, in_=sd)
                    mb_t = tpool.tile([128, 4], F32, tag="ln_mb")
                    mb = mb_t[:, :gw]
                    nc.vector.tensor_tensor(out=mb, in0=mu, in1=rstd, op=AX.mult)
                    nc.vector.tensor_scalar_mul(out=mb, in0=mb, scalar1=-1.0)
                    for wq in range(gw):
                        nc.scalar.activation(
                            out=X_t[:, wq * 128 : (wq + 1) * 128],
                            in_=X_t[:, wq * 128 : (wq + 1) * 128],
                            func=AF.Identity,
                            scale=rstd_t[:, wq : wq + 1],
                            bias=mb_t[:, wq : wq + 1],
                        )
                    nc.vector.tensor_tensor(out=_r3(Y, 128), in0=_r3(X, 128), in1=_bcast_mid(g1r, gw), op=AX.mult)
                    nc.vector.tensor_tensor(out=_r3(X, 128), in0=_r3(Y, 128), in1=_bcast_mid(b1r, gw), op=AX.add)
                    nc.scalar.activation(out=hB[:, sl], in_=X, func=AF.Relu)
                # xws1 = dis * (hB @ conv_w.T) for this chunk's windows
                for w in range(w0c, w1c):
                    tp = psA.tile([128, 128], F32, tag="ps128")
                    nc.tensor.transpose(tp[:], hB[:, ws(w)], identf[:])
                    ht = hpool.tile([128, 128], BF, tag="ht")
                    nc.scalar.copy(out=ht[:], in_=tp[:])
                    xp = psA.tile([128, 128], F32, tag="ps128")
                    nc.tensor.matmul(xp[:], lhsT=ht[:], rhs=cw[:], start=True, stop=True)
                    nc.scalar.activation(
                        out=xws1[:, ws(w)], in_=xp[:], func=AF.Copy, scale=dic[:, w : w + 1]
                    )
                csz = CHUNK_SZ[c]
                lx = dpool.tile([csz, H], BF, tag=f"lx{c}", name=f"lx{c}")
                nc.sync.dma_start(
                    out=lx[:].rearrange("(p w) f -> p w f", w=w1c - w0c),
                    in_=_r3(xws1[:, w0c * 128 : w1c * 128], 128),
                )
                sup = SUP_OF_CHUNK[c]
                dst = ctx["tb"][sup][:][TB_OFF[c] : TB_OFF[c] + 8 * csz, :]
                nc.gpsimd.collective_compute(
                    "AllGather",
                    AX.bypass,
                    replica_groups=[list(range(NCORES))],
                    ins=[lx.opt()],
                    outs=[dst],
                )

            def call_tile(s_step, s, g, h):
                ctx = ctxs[s_step]
                key = (s, g, h)
                if key not in ctx["call_tiles"]:
                    h0_, h1_, off = halves[(s, g)][h]
                    nb = h1_ - h0_
                    if nb == 0:
                        ctx["call_tiles"][key] = None
                    else:
                        gt = gpool.tile([128, gmax * H], BF, tag="gath")
                        nc.gpsimd.dma_gather(
                            gt[:, : nb * H].rearrange("p (b e) -> p b e", e=H),
                            ctx["tb"][s][:],
                            idx_sb[s][:, off : off + nb * 8],
                            nb * 128,
                            nb * 128,
                            H,
                            single_packet=False,
                            queue_num=s * 2 + h,
                        )
                        ctx["call_tiles"][key] = gt
                return ctx["call_tiles"][key]

            def prefetch(s_step, grp):
                for gg in range(grp, min(grp + 2, NGRP)):
                    w_tile(s_step, gg)
                for gg in range(grp, min(grp + 3, NGRP)):
                    for s in range(NSUP):
                        call_tile(s_step, s, gg, 0)
                        call_tile(s_step, s, gg, 1)

            def w_tile(s_step, grp):
                """Per-group selection-matrix tile, streamed from DRAM."""
                ctx = ctxs[s_step]
                if grp not in ctx["w_tiles"]:
                    b0 = int(woff[grp * 4])
                    b1 = int(woff[min(grp * 4 + 4, NW)])
                    wt = wpool.tile([128, wgmax * 128], BF, tag="W")
                    nc.sync.dma_start(
                        out=wt[:, : (b1 - b0) * 128],
                        in_=wtab_t[:, b0 * 128 : b1 * 128],
                    )
                    ctx["w_tiles"][grp] = (wt, b0)
                return ctx["w_tiles"][grp]

            def emit_groups(s_step, glo, ghi):
                ctx = ctxs[s_step]
                state = hB if s_step == 1 else h0
                xws_s = xws1 if s_step == 1 else xws0
                hdst = hA
                for grp in range(glo, ghi):
                    bg = grp * 4
                    prefetch(s_step, grp)
                    gw = min(4, NW - bg)
                    pm = psM.tile([128, 4 * 128], F32, tag="msg")
                    for wq in range(gw):
                        w = bg + wq
                        dst = pm[:, wq * 128 : (wq + 1) * 128]
                        nc.tensor.matmul(dst, lhsT=identb[:], rhs=xws_s[:, ws(w)], start=True, stop=False)
                        nblk = int(B[w].sum())
                        bi = 0
                        for s in range(NSUP):
                            for _ in range(int(B[w, s])):
                                gidx = ctx["blkpos"][s]
                                hh = halves[(s, grp)]
                                h = 0 if gidx < hh[0][1] else 1
                                h0_, h1_, _off = hh[h]
                                ct = call_tile(s_step, s, grp, h)
                                loc = gidx - h0_
                                wt_, wbase = w_tile(s_step, grp)
                                jj = gidx - int(pw[w, s])
                                wloc = int(woff[w]) - wbase + (jj if s == 0 else int(B[w, 0]) + jj)
                                nc.tensor.matmul(
                                    dst,
                                    lhsT=wt_[:, wloc * 128 : (wloc + 1) * 128],
                                    rhs=ct[:].rearrange("p (b e) -> p b e", e=H)[:, loc, :],
                                    start=False,
                                    stop=(bi == nblk - 1),
                                )
                                ctx["blkpos"][s] += 1
                                bi += 1
                    sl = slice(bg * 128, (bg + gw) * 128)
                    E1_t = tpool.tile([128, 4 * 128], F32, tag="ep_E1")
                    E1 = E1_t[:, : gw * 128]
                    E2_t = tpool.tile([128, 4 * 128], F32, tag="ep_E2")
                    E2 = E2_t[:, : gw * 128]
                    E3_t = tpool.tile([128, 4 * 128], F32, tag="ep_E3")
                    E3 = E3_t[:, : gw * 128]
                    nc.vector.tensor_tensor(
                        out=_r3(E1, 128), in0=_r3(state[:, sl], 128), in1=_bcast_mid(rootr, gw), op=AX.add
                    )
                    for wq in range(gw):
                        w = bg + wq
                        nc.scalar.activation(
                            out=E2_t[:, wq * 128 : (wq + 1) * 128],
                            in_=E1_t[:, wq * 128 : (wq + 1) * 128],
                            func=AF.Relu,
                            scale=dvc[:, w : w + 1],
                        )
                        nc.scalar.activation(
                            out=E3_t[:, wq * 128 : (wq + 1) * 128],
                            in_=pm[:, wq * 128 : (wq + 1) * 128],
                            func=AF.Copy,
                            scale=dic[:, w : w + 1],
                        )
                    nc.vector.tensor_tensor(out=E2, in0=E3, in1=E2, op=AX.add)
                    if s_step == 0:
                        # rank-1 lin-bias message term: + q_t * b2
                        for wq in range(gw):
                            w = bg + wq
                            nc.scalar.activation(
                                out=E3_t[:, wq * 128 : (wq + 1) * 128],
                                in_=b2r,
                                func=AF.Copy,
                                scale=qcl[:, w : w + 1],
                            )
                        nc.vector.tensor_tensor(out=E2, in0=E3, in1=E2, op=AX.add)
                    nc.vector.tensor_tensor(
                        out=_r3(hdst[:, sl], 128), in0=_r3(E2, 128), in1=_bcast_mid(convbr, gw), op=AX.add
                    )
                    if s_step == 1:
                        # stream the finished group straight out (contiguous
                        # per-partition rows; H == 128 makes layouts identical)
                        nc.sync.dma_start(out=out_ext[:, sl], in_=hdst[:, sl])

            # software-pipelined emission: step-1 publishes overlap step-0 consumption
            for c in range(NCHUNK):
                emit_publish0(c)
            emit_groups(0, 0, 3)
            emit_publish(0)
            emit_groups(0, 3, 6)
            emit_publish(1)
            emit_groups(0, 6, 9)
            emit_publish(2)
            emit_groups(0, 9, NGRP)
            emit_publish(3)
            emit_groups(1, 0, NGRP)
    nc.compile()
    return nc


def _rep(v):
    return np.tile(np.asarray(v, np.float32).reshape(1, H), (128, 1))


def kernel_with_results(**inputs):
    in_feat = np.asarray(inputs["in_feat"], np.float32)
    row = np.asarray(inputs["row"]).astype(np.int64)
    col = np.asarray(inputs["col"]).astype(np.int64)
    lin_w = np.asarray(inputs["lin_w"], np.float32)
    lin_b = np.asarray(inputs["lin_b"], np.float32)
    conv_w = np.asarray(inputs["conv_w"], np.float32)
    conv_b = np.asarray(inputs["conv_b"], np.float32)
    root_emb = np.asarray(inputs["root_emb"], np.float32)
    ln_gamma = np.asarray(inputs["ln_gamma"], np.float32)
    ln_beta = np.asarray(inputs["ln_beta"], np.float32)

    g = _prep_graph(row, col)
    nc = _build(g["B"], g["gpos"], g["NBLK"], g["woff"])

    ift_t = in_feat.T  # [IN, N]
    # fused step-0 weights: xw0 = X @ W2 + b2, W2 = (conv_w @ lin_w).T
    w2 = (conv_w.astype(np.float64) @ lin_w.astype(np.float64)).astype(np.float32)
    b2 = (conv_w.astype(np.float64) @ lin_b.astype(np.float64)).astype(np.float32)
    consts = np.concatenate(
        [_rep(lin_b), _rep(root_emb[0]), _rep(conv_b), _rep(ln_gamma[1]), _rep(ln_beta[1]), _rep(b2)],
        axis=1,
    )
    lin_wT = np.ascontiguousarray(lin_w.T).astype(BF16)
    w2T = np.ascontiguousarray(w2.T).astype(BF16)
    conv_wT = np.ascontiguousarray(conv_w.T).astype(BF16)

    in_maps = []
    for k in range(NCORES):
        ift_k = np.zeros((IN, PADN), BF16)
        ift_k[:, :NPC] = ift_t[:, k * NPC : (k + 1) * NPC].astype(BF16)
        m = {
            "iftl": ift_k,
            "lin_wT": lin_wT,
            "w2T": w2T,
            "conv_wT": conv_wT,
            "consts": consts,
            "wtab": g["wtab"][k],
            "discols": g["dis_cols"][k],
            "dinvcols": g["dinv_cols"][k],
            "qcols": g["q_cols"][k],
        }
        for s in range(NSUP):
            m[f"idx{s}"] = g["idx_w"][s][k]
        in_maps.append(m)

    res = run_bass_kernel_spmd(nc, in_maps, list(range(NCORES)))
    shards = []
    for k in range(NCORES):
        o = np.asarray(res.results[k]["out"])  # [128, NW*H], [p, w*H+f]
        o = o.reshape(128, NW, H).transpose(1, 0, 2).reshape(PADN, H)
        shards.append(o[:NPC])
    out = np.concatenate(shards, axis=0)
    return out.astype(np.float32), res


def kernel(**inputs):
    out, _ = kernel_with_results(**inputs)
    return out
